# revision 21
# baseline (speedup 1.0000x reference)
"""Trainium2 Bass kernel for nn_LongTermMemoryMLP.

Per-batch-weight 3-layer MLP:
    h0 = relu(q @ W0^T + b0); h1 = relu(h0 @ W1^T + b1); out = h1 @ W2^T + b2
with q: [B,S,DIN], W0: [B,DH,DIN], W1: [B,DH,DH], W2: [B,DOUT,DH], B=8.

Sharding: data-parallel over batch — one batch sample (and its weight slabs)
per NeuronCore, 8 cores, no cross-core communication.

Device-side strategy: activations are kept feature-major ([feature, seq],
feature on partitions) so every layer is a plain accumulated matmul with the
(pre-transposed) weights as the stationary operand and the activations as the
moving operand — no on-chip transposes. The final layer flips orientation
(stationary = activation tile, moving = W2^T) so the output lands seq-major
and can be DMA'd out contiguously. Inputs are pre-transposed AND pre-cast to
bf16 on the host: bf16 streams at the PE's full 1 row/cycle (216 ns measured
per 128x128x512 matmul, the warm roofline) and halves all input DMA traffic.
fp8 was evaluated and rejected: DoubleRow e4m3 measures exactly 2x bf16 on
this hw (218.8 ns for a 2-k-tile matmul), but uncompensated e4m3 gives
3.7e-2..6.9e-2 end-to-end rel err (gate 2e-2) and the 3-pass hi/lo
compensation that fixes it costs 1.5x bf16 — a net loss. So the bf16 PE
roofline (~218.5 us for 1024 matmuls) is the floor and everything else is
startup/tail/gap engineering:

- Every input tensor is packed on the host so each DMA is a dense
  per-partition-contiguous 2D block ([128, F] with F contiguous): q is
  chunk-major [NCH, 128, K0*SC], weights are [128, K*out] k-major slabs.
  This minimizes HWDGE descriptor count (a q-chunk issue drops from
  ~1.5 us to ~0.6 us of sync-engine time), so the sync-queue issue of the
  startup loads (w0a q0a w0b q0b b0 q1 | w1 b1 | w2 b2, one FIFO ring in
  consumption order) completes fast enough that the ring never starves.
- Chunk 0's L0 is emitted k-split: each m-group's accumulation is two
  k-pair passes with the PSUM group left open in between, interleaved
  over 4 m-groups, and q0 arrives as two k-half DMAs. Pass A (k=0,1)
  then waits only on w0a + q0[k<2] (1.0 MiB) instead of the full
  w0+q0 (2 MiB), starting the real stream ~1.5 us earlier. All matmuls
  stay full-width N=512: variants using N=128 strips or narrow warmups
  left the whole run at the mid p-state (292 us vs 241 — the clock
  governor keys on sustained full-width array activity).
- The PE clock (DVFS) ramps only while the PE is busy, so dummy warmup
  matmuls on garbage data spin it up while the startup DMAs land. Warmup
  is sized to end right as pass A's data arrives: oversizing it delays
  the real stream; a short idle before the first real matmul is harmless
  (no clock decay observed at ~1.3 us idle).
- Warmup operand memsets run on GpSimd (idle at preamble end) so the PE
  starts ~0.9 us earlier than with DVE memsets.
- PSUM banks 4/2/2 (L0/L1/L2): consecutive L0(c)+L0(c+1) phases of the
  software pipeline emit 16 back-to-back 4-matmul groups (864 ns each)
  whose consumer (scalar relu, ~700 ns) barely keeps up; with only 3 L0
  banks the bank recycle stalls the PE ~0.8-1.2 us per run. L1/L2 groups
  are 8 matmuls (1728 ns) — 2 banks each drain comfortably.
- The output is stored bf16 (halves the output DMA and the strictly-serial
  tail after the last matmul) and widened to fp32 on the host. The final
  seq-tile's PSUM->add->DMA chain is quartered across both HWDGE rings.
  Accumulation stays fp32 in PSUM; measured end-to-end relative error is
  4.7e-3 against the fp32 reference, vs the 2e-2 gate.

Software pipeline: emit L0 of chunk c+1 ahead of L1/L2 of chunk c, so the
matmul stream never depends on a DMA issued less than a full chunk earlier,
and the 2 MiB w1 slab has landed before L1(c0) needs it.
"""

import numpy as np

import ml_dtypes

import concourse.bass as bass
import concourse.tile as tile
from concourse import bacc, mybir
from concourse.bass_utils import run_bass_kernel_spmd

B, S, DIN, DH, DOUT = 8, 4096, 512, 1024, 512
SC = 512  # seq chunk processed per pipeline iteration

BF16 = mybir.dt.bfloat16
F32 = mybir.dt.float32

K0 = DIN // 128   # 4  k-tiles, layer 0
K1 = DH // 128    # 8  k-tiles, layers 1/2
M0 = DH // 128    # 8  m-tiles (feature tiles of h0/h1)
MT = SC // 128    # 4  seq m-tiles per chunk, layer 2
NCH = S // SC     # 8  chunks

N_WARM = 10


def build_nc():
    nc = bacc.Bacc("TRN2")
    # Host-packed dense layouts: every DMA below reads a per-partition
    # contiguous [128, F] block (minimal descriptors, fast issue).
    qd = nc.dram_tensor("qd", (NCH, 128, K0 * SC), BF16, kind="ExternalInput")
    w0d = nc.dram_tensor("w0d", (128, K0 * DH), BF16, kind="ExternalInput")
    w1d = nc.dram_tensor("w1d", (128, K1 * DH), BF16, kind="ExternalInput")
    w2d = nc.dram_tensor("w2d", (128, K1 * DOUT), BF16, kind="ExternalInput")
    # b0/b1 host-pre-transposed to [128, DH//128] (partition-major)
    b0 = nc.dram_tensor("b0", (128, DH // 128), F32, kind="ExternalInput")
    b1 = nc.dram_tensor("b1", (128, DH // 128), F32, kind="ExternalInput")
    b2 = nc.dram_tensor("b2", (DOUT,), F32, kind="ExternalInput")
    out = nc.dram_tensor("out", (S, DOUT), BF16, kind="ExternalOutput")

    Relu = mybir.ActivationFunctionType.Relu

    with tile.TileContext(nc) as tc:
        with (
            tc.tile_pool(name="weights", bufs=1) as wpool,
            tc.tile_pool(name="biases", bufs=1) as bpool,
            tc.tile_pool(name="acts", bufs=2) as apool,
            tc.tile_pool(name="qin", bufs=2) as qpool,
            tc.tile_pool(name="outp", bufs=4) as opool,
            tc.tile_pool(name="psum0", bufs=4, space="PSUM") as ppool0,
            tc.tile_pool(name="psum1", bufs=2, space="PSUM") as ppool1,
            tc.tile_pool(name="psum2", bufs=2, space="PSUM") as ppool2,
        ):
            # Pre-warm the PE clock with dummy matmuls on garbage data
            # while the startup DMAs land: the real matmul stream then
            # starts near 2.4 GHz. Full-width N=512 matmuls are required —
            # a variant with 40 tiny N=128 warmups left the whole run at
            # the mid p-state (292 us vs 241), so the governor appears to
            # key on sustained full-width array activity. GpSimd memsets:
            # it is the first engine free after the preamble, so the PE
            # starts ~0.9 us earlier than with DVE memsets.
            g_lhs = apool.tile([128, 128], BF16, tag="warm_lhs")
            g_rhs = apool.tile([128, SC], BF16, tag="warm_rhs")
            nc.gpsimd.memset(g_lhs, 0.0)
            nc.gpsimd.memset(g_rhs, 0.0)
            warm_ps = ppool0.tile([128, SC], F32, tag="ps0")
            for i in range(N_WARM):
                nc.tensor.matmul(
                    warm_ps, lhsT=g_lhs, rhs=g_rhs,
                    start=(i == 0), stop=(i == N_WARM - 1),
                )

            # Startup loads: ALL on the sync HWDGE ring, in the order the
            # PE consumes them (w0 q0-quarters b0 q1 | w1 b1 | w2 b2). One
            # queue streaming large DMAs sustains ~341-365 GB/s, while
            # splitting the same bytes across parallel rings drains slower
            # aggregate — and FIFO order on one ring guarantees w0/q0 are
            # never slowed by the later, slack-rich w1/w2 transfers.
            w0a_sb = wpool.tile([128, K0 // 2, DH], BF16, tag="w0a")
            w0b_sb = wpool.tile([128, K0 // 2, DH], BF16, tag="w0b")
            b0_sb = bpool.tile([128, M0], F32, tag="b0")
            b1_sb = bpool.tile([128, M0], F32, tag="b1")
            nc.sync.dma_start(out=w0a_sb, in_=w0d[:, 0:(K0 // 2) * DH])

            def w0_slice(k, m):
                t = w0a_sb if k < K0 // 2 else w0b_sb
                return t[:, k % (K0 // 2), m * 128:(m + 1) * 128]

            def load_q(c):
                t = qpool.tile([128, K0, SC], BF16, tag="q", name=f"q{c}")
                nc.sync.dma_start(out=t, in_=qd[c])
                return t

            # q0 in two k-half DMAs: pass A of the k-split below then waits
            # only on w0a + q0[k=0,1] (1.0 MiB) instead of the full q0.
            q0_sb = qpool.tile([128, K0, SC], BF16, tag="q", name="q0")
            nc.sync.dma_start(
                out=q0_sb[:, 0:K0 // 2, :], in_=qd[0][:, 0:(K0 // 2) * SC]
            )
            nc.sync.dma_start(out=w0b_sb, in_=w0d[:, (K0 // 2) * DH:K0 * DH])
            nc.sync.dma_start(
                out=q0_sb[:, K0 // 2:K0, :], in_=qd[0][:, (K0 // 2) * SC:K0 * SC]
            )
            nc.sync.dma_start(out=b0_sb, in_=b0[:, :])
            q1_sb = load_q(1)

            w1_sb = wpool.tile([128, K1, DH], BF16, tag="w1")
            nc.sync.dma_start(out=w1_sb, in_=w1d[:, :])
            nc.sync.dma_start(out=b1_sb, in_=b1[:, :])

            def w1_slice(k, m):
                return w1_sb[:, k, m * 128:(m + 1) * 128]

            w2_sb = wpool.tile([128, K1, DOUT], BF16, tag="w2")
            nc.sync.dma_start(out=w2_sb, in_=w2d[:, :])
            b2_sb = bpool.tile([128, DOUT], F32, tag="b2")
            b2_ap = b2[:]
            b2_bcast = bass.AP(
                tensor=b2_ap.tensor,
                offset=b2_ap.offset,
                ap=[[0, 128]] + [list(d) for d in b2_ap.ap],
            )
            nc.sync.dma_start(out=b2_sb, in_=b2_bcast)

            def layer0(c, q_sb):
                h0_sb = []
                for m in range(M0):
                    ps = ppool0.tile([128, SC], F32, tag="ps0", name=f"ps0_{c}_{m}")
                    for k in range(K0):
                        nc.tensor.matmul(
                            ps,
                            lhsT=w0_slice(k, m),
                            rhs=q_sb[:, k, :],
                            start=(k == 0),
                            stop=(k == K0 - 1),
                        )
                    h = apool.tile([128, SC], BF16, tag=f"h0_{m}", name=f"h0_{c}_{m}")
                    nc.scalar.activation(h, ps, Relu, bias=b0_sb[:, m:m + 1])
                    h0_sb.append(h)
                return h0_sb

            def layer0_c0(q_sb):
                # First chunk, k-split: each m-group's accumulation is
                # emitted as two k-pair passes with the PSUM group left
                # open in between, interleaved over 4 m-groups (= the 4
                # L0 banks). Pass A (k=0,1) needs only w0a+q0 (1.5 MiB,
                # ready ~12.3 us); w0b lands (~14 us) while pass A runs.
                # Full-width N=512 matmuls throughout (narrow warmups or
                # strips leave the clock governor at the mid p-state).
                h0_sb = [None] * M0
                for half in range(2):
                    ms = list(range(half * 4, half * 4 + 4))
                    pss = {}
                    for phase in range(2):
                        for m in ms:
                            if phase == 0:
                                pss[m] = ppool0.tile(
                                    [128, SC], F32, tag="ps0", name=f"ps0_0_{m}"
                                )
                            ps = pss[m]
                            for k in (phase * 2, phase * 2 + 1):
                                nc.tensor.matmul(
                                    ps,
                                    lhsT=w0_slice(k, m),
                                    rhs=q_sb[:, k, :],
                                    start=(k == 0),
                                    stop=(k == K0 - 1),
                                    skip_group_check=True,
                                )
                            if phase == 1:
                                h = apool.tile(
                                    [128, SC], BF16, tag=f"h0_{m}",
                                    name=f"h0_0_{m}",
                                )
                                nc.scalar.activation(
                                    h, ps, Relu, bias=b0_sb[:, m:m + 1]
                                )
                                h0_sb[m] = h
                return h0_sb

            def layer1(c, h0_sb):
                h1_sb = []
                for m in range(M0):
                    ps = ppool1.tile([128, SC], F32, tag="ps1", name=f"ps1_{c}_{m}")
                    for k in range(K1):
                        nc.tensor.matmul(
                            ps,
                            lhsT=w1_slice(k, m),
                            rhs=h0_sb[k],
                            start=(k == 0),
                            stop=(k == K1 - 1),
                        )
                    h = apool.tile([128, SC], BF16, tag=f"h1_{m}", name=f"h1_{c}_{m}")
                    nc.scalar.activation(h, ps, Relu, bias=b1_sb[:, m:m + 1])
                    h1_sb.append(h)
                return h1_sb

            def layer2(c, h1_sb):
                s0 = c * SC
                last = c == NCH - 1
                for mt in range(MT):
                    ps = ppool2.tile([128, DOUT], F32, tag="ps2", name=f"ps2_{c}_{mt}")
                    for k in range(K1):
                        nc.tensor.matmul(
                            ps,
                            lhsT=h1_sb[k][:, mt * 128:(mt + 1) * 128],
                            rhs=w2_sb[:, k, :],
                            start=(k == 0),
                            stop=(k == K1 - 1),
                        )
                    ot = opool.tile([128, DOUT], BF16, tag="ot", name=f"ot_{c}_{mt}")
                    r0 = s0 + mt * 128
                    if last and mt == MT - 1:
                        # Tail trim: halve the strictly-serial
                        # PSUM->add->DMA chain after the very last matmul —
                        # one half per HWDGE ring, so the two ~600ns issues
                        # run in parallel instead of 2-deep per engine
                        # (quarters serialized ~2x600ns on each ring).
                        Q = DOUT // 2
                        for i in range(2):
                            sl = slice(i * Q, (i + 1) * Q)
                            nc.vector.tensor_add(ot[:, sl], ps[:, sl], b2_sb[:, sl])
                            eng = nc.scalar if i % 2 == 0 else nc.sync
                            eng.dma_start(out=out[r0:r0 + 128, sl], in_=ot[:, sl])
                    else:
                        nc.vector.tensor_add(ot, ps, b2_sb)
                        eng = nc.scalar if mt % 2 == 0 else nc.sync
                        eng.dma_start(out=out[r0:r0 + 128, :], in_=ot)

            # Software pipeline: emit L0 of chunk c+1 ahead of L1/L2 of
            # chunk c, so the matmul stream never depends on a DMA issued
            # less than a full chunk earlier.
            h0_cur = layer0_c0(q0_sb)
            for c in range(NCH):
                h0_next = None
                if c + 1 < NCH:
                    h0_next = layer0(c + 1, q1_sb if c == 0 else load_q(c + 1))
                layer2(c, layer1(c, h0_cur))
                h0_cur = h0_next
    nc.finalize()
    return nc


_NC = None


def _get_nc():
    global _NC
    if _NC is None:
        _NC = build_nc()
    return _NC


def make_in_maps(inputs):
    bf16 = ml_dtypes.bfloat16
    q, W0, b0, W1, b1, W2, b2 = (
        inputs["query"], inputs["W0"], inputs["b0"], inputs["W1"],
        inputs["b1"], inputs["W2"], inputs["b2"],
    )
    in_maps = []
    for b in range(B):
        # qd[c, p, k*SC + s] = q[b, c*SC+s, k*128+p]
        qT = np.asarray(q[b]).T.astype(bf16)            # [DIN, S]
        qd = (
            qT.reshape(K0, 128, NCH, SC)
            .transpose(2, 1, 0, 3)
            .reshape(NCH, 128, K0 * SC)
        )

        # wXd[p, k*out + j] = WX[b].T[(k*128+p), j]
        w0t = np.asarray(W0[b]).T.astype(bf16)          # [DIN, DH]
        w0d = w0t.reshape(K0, 128, DH).transpose(1, 0, 2).reshape(128, K0 * DH)
        w1t = np.asarray(W1[b]).T.astype(bf16)          # [DH, DH]
        w1d = w1t.reshape(K1, 128, DH).transpose(1, 0, 2).reshape(128, K1 * DH)
        w2t = np.asarray(W2[b]).T.astype(bf16)          # [DH, DOUT]
        w2d = w2t.reshape(K1, 128, DOUT).transpose(1, 0, 2).reshape(128, K1 * DOUT)
        in_maps.append({
            "qd": np.ascontiguousarray(qd),
            "w0d": np.ascontiguousarray(w0d),
            "w1d": np.ascontiguousarray(w1d),
            "w2d": np.ascontiguousarray(w2d),
            "b0": np.ascontiguousarray(
                np.asarray(b0[b], dtype=np.float32).reshape(DH // 128, 128).T
            ),
            "b1": np.ascontiguousarray(
                np.asarray(b1[b], dtype=np.float32).reshape(DH // 128, 128).T
            ),
            "b2": np.asarray(b2[b], dtype=np.float32),
        })
    return in_maps


def run(inputs, trace=False):
    nc = _get_nc()
    in_maps = make_in_maps(inputs)
    res = run_bass_kernel_spmd(nc, in_maps, core_ids=list(range(B)), trace=trace)
    out = np.stack(
        [np.asarray(r["out"]).astype(np.float32) for r in res.results]
    )
    return out, res


def kernel(**inputs) -> np.ndarray:
    out, _ = run(inputs, trace=False)
    return out


# revision 22
# speedup vs baseline: 1.0042x; 1.0042x over previous
"""Trainium2 Bass kernel for nn_LongTermMemoryMLP.

Per-batch-weight 3-layer MLP:
    h0 = relu(q @ W0^T + b0); h1 = relu(h0 @ W1^T + b1); out = h1 @ W2^T + b2
with q: [B,S,DIN], W0: [B,DH,DIN], W1: [B,DH,DH], W2: [B,DOUT,DH], B=8.

Sharding: data-parallel over batch — one batch sample (and its weight slabs)
per NeuronCore, 8 cores, no cross-core communication.

Device-side strategy: activations are kept feature-major ([feature, seq],
feature on partitions) so every layer is a plain accumulated matmul with the
(pre-transposed) weights as the stationary operand and the activations as the
moving operand — no on-chip transposes. The final layer flips orientation
(stationary = activation tile, moving = W2^T) so the output lands seq-major
and can be DMA'd out contiguously. Inputs are pre-transposed AND pre-cast to
bf16 on the host: bf16 streams at the PE's full 1 row/cycle (216 ns measured
per 128x128x512 matmul, the warm roofline) and halves all input DMA traffic.
fp8 was evaluated and rejected: DoubleRow e4m3 measures exactly 2x bf16 on
this hw (218.8 ns for a 2-k-tile matmul), but uncompensated e4m3 gives
3.7e-2..6.9e-2 end-to-end rel err (gate 2e-2) and the 3-pass hi/lo
compensation that fixes it costs 1.5x bf16 — a net loss. So the bf16 PE
roofline (~218.5 us for 1024 matmuls) is the floor and everything else is
startup/tail/gap engineering:

- Every input tensor is packed on the host so each DMA is a dense
  per-partition-contiguous 2D block ([128, F] with F contiguous): q is
  chunk-major [NCH, 128, K0*SC], weights are [128, K*out] k-major slabs.
  This minimizes HWDGE descriptor count (a q-chunk issue drops from
  ~1.5 us to ~0.6 us of sync-engine time), so the sync-queue issue of the
  startup loads (w0a q0a w0b q0b b0 q1 | w1 b1 | w2 b2, one FIFO ring in
  consumption order) completes fast enough that the ring never starves.
- Chunk 0's L0 is emitted k-split: each m-group's accumulation is two
  k-pair passes with the PSUM group left open in between, interleaved
  over 4 m-groups, and q0 arrives as two k-half DMAs. Pass A (k=0,1)
  then waits only on w0a + q0[k<2] (1.0 MiB) instead of the full
  w0+q0 (2 MiB), starting the real stream ~1.5 us earlier. All matmuls
  stay full-width N=512: variants using N=128 strips or narrow warmups
  left the whole run at the mid p-state (292 us vs 241 — the clock
  governor keys on sustained full-width array activity).
- The PE clock (DVFS) ramps only while the PE is busy, so dummy warmup
  matmuls on garbage data spin it up while the startup DMAs land. Warmup
  is sized to end right as pass A's data arrives: oversizing it delays
  the real stream; a short idle before the first real matmul is harmless
  (no clock decay observed at ~1.3 us idle).
- Warmup operand memsets run on GpSimd (idle at preamble end) so the PE
  starts ~0.9 us earlier than with DVE memsets.
- PSUM banks 4/2/2 (L0/L1/L2): consecutive L0(c)+L0(c+1) phases of the
  software pipeline emit 16 back-to-back 4-matmul groups (864 ns each)
  whose consumer (scalar relu, ~700 ns) barely keeps up; with only 3 L0
  banks the bank recycle stalls the PE ~0.8-1.2 us per run. L1/L2 groups
  are 8 matmuls (1728 ns) — 2 banks each drain comfortably.
- The output is stored bf16 (halves the output DMA and the strictly-serial
  tail after the last matmul) and widened to fp32 on the host. The final
  seq-tile's PSUM->add->DMA chain is quartered across both HWDGE rings.
  Accumulation stays fp32 in PSUM; measured end-to-end relative error is
  4.7e-3 against the fp32 reference, vs the 2e-2 gate.

Software pipeline: emit L0 of chunk c+1 ahead of L1/L2 of chunk c, so the
matmul stream never depends on a DMA issued less than a full chunk earlier,
and the 2 MiB w1 slab has landed before L1(c0) needs it.
"""

import numpy as np

import ml_dtypes

import concourse.bass as bass
import concourse.tile as tile
from concourse import bacc, mybir
from concourse.bass_utils import run_bass_kernel_spmd

B, S, DIN, DH, DOUT = 8, 4096, 512, 1024, 512
SC = 512  # seq chunk processed per pipeline iteration

BF16 = mybir.dt.bfloat16
F32 = mybir.dt.float32

K0 = DIN // 128   # 4  k-tiles, layer 0
K1 = DH // 128    # 8  k-tiles, layers 1/2
M0 = DH // 128    # 8  m-tiles (feature tiles of h0/h1)
MT = SC // 128    # 4  seq m-tiles per chunk, layer 2
NCH = S // SC     # 8  chunks

N_WARM = 10


def build_nc():
    nc = bacc.Bacc("TRN2")
    # Host-packed dense layouts: every DMA below reads a per-partition
    # contiguous [128, F] block (minimal descriptors, fast issue).
    qd = nc.dram_tensor("qd", (NCH, 128, K0 * SC), BF16, kind="ExternalInput")
    w0d = nc.dram_tensor("w0d", (128, K0 * DH), BF16, kind="ExternalInput")
    w1d = nc.dram_tensor("w1d", (128, K1 * DH), BF16, kind="ExternalInput")
    w2d = nc.dram_tensor("w2d", (128, K1 * DOUT), BF16, kind="ExternalInput")
    # b0/b1 host-pre-transposed to [128, DH//128] (partition-major)
    b0 = nc.dram_tensor("b0", (128, DH // 128), F32, kind="ExternalInput")
    b1 = nc.dram_tensor("b1", (128, DH // 128), F32, kind="ExternalInput")
    b2 = nc.dram_tensor("b2", (DOUT,), F32, kind="ExternalInput")
    out = nc.dram_tensor("out", (S, DOUT), BF16, kind="ExternalOutput")

    Relu = mybir.ActivationFunctionType.Relu

    with tile.TileContext(nc) as tc:
        with (
            tc.tile_pool(name="weights", bufs=1) as wpool,
            tc.tile_pool(name="biases", bufs=1) as bpool,
            tc.tile_pool(name="acts", bufs=2) as apool,
            tc.tile_pool(name="qin", bufs=2) as qpool,
            tc.tile_pool(name="outp", bufs=4) as opool,
            tc.tile_pool(name="psum0", bufs=4, space="PSUM") as ppool0,
            tc.tile_pool(name="psum1", bufs=2, space="PSUM") as ppool1,
            tc.tile_pool(name="psum2", bufs=2, space="PSUM") as ppool2,
        ):
            # Pre-warm the PE clock with dummy matmuls on garbage data
            # while the startup DMAs land: the real matmul stream then
            # starts near 2.4 GHz. Full-width N=512 matmuls are required —
            # a variant with 40 tiny N=128 warmups left the whole run at
            # the mid p-state (292 us vs 241), so the governor appears to
            # key on sustained full-width array activity. GpSimd memsets:
            # it is the first engine free after the preamble, so the PE
            # starts ~0.9 us earlier than with DVE memsets.
            g_lhs = apool.tile([128, 128], BF16, tag="warm_lhs")
            g_rhs = apool.tile([128, SC], BF16, tag="warm_rhs")
            nc.gpsimd.memset(g_lhs, 0.0)
            nc.gpsimd.memset(g_rhs, 0.0)
            warm_ps = ppool0.tile([128, SC], F32, tag="ps0")
            for i in range(N_WARM):
                nc.tensor.matmul(
                    warm_ps, lhsT=g_lhs, rhs=g_rhs,
                    start=(i == 0), stop=(i == N_WARM - 1),
                )

            # Startup loads: ALL on the sync HWDGE ring, in the order the
            # PE consumes them (w0 q0-quarters b0 q1 | w1 b1 | w2 b2). One
            # queue streaming large DMAs sustains ~341-365 GB/s, while
            # splitting the same bytes across parallel rings drains slower
            # aggregate — and FIFO order on one ring guarantees w0/q0 are
            # never slowed by the later, slack-rich w1/w2 transfers.
            w0a_sb = wpool.tile([128, K0 // 2, DH], BF16, tag="w0a")
            w0b_sb = wpool.tile([128, K0 // 2, DH], BF16, tag="w0b")
            b0_sb = bpool.tile([128, M0], F32, tag="b0")
            b1_sb = bpool.tile([128, M0], F32, tag="b1")
            nc.sync.dma_start(out=w0a_sb, in_=w0d[:, 0:(K0 // 2) * DH])

            def w0_slice(k, m):
                t = w0a_sb if k < K0 // 2 else w0b_sb
                return t[:, k % (K0 // 2), m * 128:(m + 1) * 128]

            def load_q(c):
                t = qpool.tile([128, K0, SC], BF16, tag="q", name=f"q{c}")
                nc.sync.dma_start(out=t, in_=qd[c])
                return t

            # q0 in two k-half DMAs: pass A of the k-split below then waits
            # only on w0a + q0[k=0,1] (1.0 MiB) instead of the full q0.
            q0_sb = qpool.tile([128, K0, SC], BF16, tag="q", name="q0")
            nc.sync.dma_start(
                out=q0_sb[:, 0:K0 // 2, :], in_=qd[0][:, 0:(K0 // 2) * SC]
            )
            nc.sync.dma_start(out=w0b_sb, in_=w0d[:, (K0 // 2) * DH:K0 * DH])
            nc.sync.dma_start(
                out=q0_sb[:, K0 // 2:K0, :], in_=qd[0][:, (K0 // 2) * SC:K0 * SC]
            )
            nc.sync.dma_start(out=b0_sb, in_=b0[:, :])
            q1_sb = load_q(1)

            w1_sb = wpool.tile([128, K1, DH], BF16, tag="w1")
            nc.sync.dma_start(out=w1_sb, in_=w1d[:, :])
            nc.sync.dma_start(out=b1_sb, in_=b1[:, :])

            def w1_slice(k, m):
                return w1_sb[:, k, m * 128:(m + 1) * 128]

            w2_sb = wpool.tile([128, K1, DOUT], BF16, tag="w2")
            nc.sync.dma_start(out=w2_sb, in_=w2d[:, :])
            b2_sb = bpool.tile([128, DOUT], F32, tag="b2")
            b2_ap = b2[:]
            b2_bcast = bass.AP(
                tensor=b2_ap.tensor,
                offset=b2_ap.offset,
                ap=[[0, 128]] + [list(d) for d in b2_ap.ap],
            )
            nc.sync.dma_start(out=b2_sb, in_=b2_bcast)

            def layer0(c, q_sb):
                h0_sb = []
                for m in range(M0):
                    ps = ppool0.tile([128, SC], F32, tag="ps0", name=f"ps0_{c}_{m}")
                    for k in range(K0):
                        nc.tensor.matmul(
                            ps,
                            lhsT=w0_slice(k, m),
                            rhs=q_sb[:, k, :],
                            start=(k == 0),
                            stop=(k == K0 - 1),
                        )
                    h = apool.tile([128, SC], BF16, tag=f"h0_{m}", name=f"h0_{c}_{m}")
                    nc.scalar.activation(h, ps, Relu, bias=b0_sb[:, m:m + 1])
                    h0_sb.append(h)
                return h0_sb

            def layer0_c0(q_sb):
                # First chunk, k-split: each m-group's accumulation is
                # emitted as two k-pair passes with the PSUM group left
                # open in between, interleaved over 4 m-groups (= the 4
                # L0 banks). Pass A (k=0,1) needs only w0a+q0 (1.5 MiB,
                # ready ~12.3 us); w0b lands (~14 us) while pass A runs.
                # Full-width N=512 matmuls throughout (narrow warmups or
                # strips leave the clock governor at the mid p-state).
                h0_sb = [None] * M0
                for half in range(2):
                    ms = list(range(half * 4, half * 4 + 4))
                    pss = {}
                    for phase in range(2):
                        for m in ms:
                            if phase == 0:
                                pss[m] = ppool0.tile(
                                    [128, SC], F32, tag="ps0", name=f"ps0_0_{m}"
                                )
                            ps = pss[m]
                            for k in (phase * 2, phase * 2 + 1):
                                nc.tensor.matmul(
                                    ps,
                                    lhsT=w0_slice(k, m),
                                    rhs=q_sb[:, k, :],
                                    start=(k == 0),
                                    stop=(k == K0 - 1),
                                    skip_group_check=True,
                                )
                            if phase == 1:
                                h = apool.tile(
                                    [128, SC], BF16, tag=f"h0_{m}",
                                    name=f"h0_0_{m}",
                                )
                                nc.scalar.activation(
                                    h, ps, Relu, bias=b0_sb[:, m:m + 1]
                                )
                                h0_sb[m] = h
                return h0_sb

            def layer1(c, h0_sb):
                h1_sb = []
                for m in range(M0):
                    ps = ppool1.tile([128, SC], F32, tag="ps1", name=f"ps1_{c}_{m}")
                    for k in range(K1):
                        nc.tensor.matmul(
                            ps,
                            lhsT=w1_slice(k, m),
                            rhs=h0_sb[k],
                            start=(k == 0),
                            stop=(k == K1 - 1),
                        )
                    h = apool.tile([128, SC], BF16, tag=f"h1_{m}", name=f"h1_{c}_{m}")
                    nc.scalar.activation(h, ps, Relu, bias=b1_sb[:, m:m + 1])
                    h1_sb.append(h)
                return h1_sb

            def layer2(c, h1_sb):
                s0 = c * SC
                last = c == NCH - 1
                for mt in range(MT):
                    ps = ppool2.tile([128, DOUT], F32, tag="ps2", name=f"ps2_{c}_{mt}")
                    for k in range(K1):
                        nc.tensor.matmul(
                            ps,
                            lhsT=h1_sb[k][:, mt * 128:(mt + 1) * 128],
                            rhs=w2_sb[:, k, :],
                            start=(k == 0),
                            stop=(k == K1 - 1),
                        )
                    ot = opool.tile([128, DOUT], BF16, tag="ot", name=f"ot_{c}_{mt}")
                    r0 = s0 + mt * 128
                    if last and mt == MT - 1:
                        # Tail trim: quarter the strictly-serial
                        # PSUM->add->DMA chain after the very last matmul,
                        # alternating the two HWDGE rings so issue overlaps
                        # (halves were tried: 240.9 us min vs 240.3 — the
                        # longer first add outweighs the parallel issues).
                        Q = DOUT // 4
                        for i in range(4):
                            sl = slice(i * Q, (i + 1) * Q)
                            nc.vector.tensor_add(ot[:, sl], ps[:, sl], b2_sb[:, sl])
                            eng = nc.scalar if i % 2 == 0 else nc.sync
                            eng.dma_start(out=out[r0:r0 + 128, sl], in_=ot[:, sl])
                    else:
                        nc.vector.tensor_add(ot, ps, b2_sb)
                        eng = nc.scalar if mt % 2 == 0 else nc.sync
                        eng.dma_start(out=out[r0:r0 + 128, :], in_=ot)

            # Software pipeline: emit L0 of chunk c+1 ahead of L1/L2 of
            # chunk c, so the matmul stream never depends on a DMA issued
            # less than a full chunk earlier.
            h0_cur = layer0_c0(q0_sb)
            for c in range(NCH):
                h0_next = None
                if c + 1 < NCH:
                    h0_next = layer0(c + 1, q1_sb if c == 0 else load_q(c + 1))
                layer2(c, layer1(c, h0_cur))
                h0_cur = h0_next
    nc.finalize()
    return nc


_NC = None


def _get_nc():
    global _NC
    if _NC is None:
        _NC = build_nc()
    return _NC


def make_in_maps(inputs):
    bf16 = ml_dtypes.bfloat16
    q, W0, b0, W1, b1, W2, b2 = (
        inputs["query"], inputs["W0"], inputs["b0"], inputs["W1"],
        inputs["b1"], inputs["W2"], inputs["b2"],
    )
    in_maps = []
    for b in range(B):
        # qd[c, p, k*SC + s] = q[b, c*SC+s, k*128+p]
        qT = np.asarray(q[b]).T.astype(bf16)            # [DIN, S]
        qd = (
            qT.reshape(K0, 128, NCH, SC)
            .transpose(2, 1, 0, 3)
            .reshape(NCH, 128, K0 * SC)
        )

        # wXd[p, k*out + j] = WX[b].T[(k*128+p), j]
        w0t = np.asarray(W0[b]).T.astype(bf16)          # [DIN, DH]
        w0d = w0t.reshape(K0, 128, DH).transpose(1, 0, 2).reshape(128, K0 * DH)
        w1t = np.asarray(W1[b]).T.astype(bf16)          # [DH, DH]
        w1d = w1t.reshape(K1, 128, DH).transpose(1, 0, 2).reshape(128, K1 * DH)
        w2t = np.asarray(W2[b]).T.astype(bf16)          # [DH, DOUT]
        w2d = w2t.reshape(K1, 128, DOUT).transpose(1, 0, 2).reshape(128, K1 * DOUT)
        in_maps.append({
            "qd": np.ascontiguousarray(qd),
            "w0d": np.ascontiguousarray(w0d),
            "w1d": np.ascontiguousarray(w1d),
            "w2d": np.ascontiguousarray(w2d),
            "b0": np.ascontiguousarray(
                np.asarray(b0[b], dtype=np.float32).reshape(DH // 128, 128).T
            ),
            "b1": np.ascontiguousarray(
                np.asarray(b1[b], dtype=np.float32).reshape(DH // 128, 128).T
            ),
            "b2": np.asarray(b2[b], dtype=np.float32),
        })
    return in_maps


def run(inputs, trace=False):
    nc = _get_nc()
    in_maps = make_in_maps(inputs)
    res = run_bass_kernel_spmd(nc, in_maps, core_ids=list(range(B)), trace=trace)
    out = np.stack(
        [np.asarray(r["out"]).astype(np.float32) for r in res.results]
    )
    return out, res


def kernel(**inputs) -> np.ndarray:
    out, _ = run(inputs, trace=False)
    return out


# revision 23
# speedup vs baseline: 1.0070x; 1.0027x over previous
"""Trainium2 Bass kernel for nn_LongTermMemoryMLP.

Per-batch-weight 3-layer MLP:
    h0 = relu(q @ W0^T + b0); h1 = relu(h0 @ W1^T + b1); out = h1 @ W2^T + b2
with q: [B,S,DIN], W0: [B,DH,DIN], W1: [B,DH,DH], W2: [B,DOUT,DH], B=8.

Sharding: data-parallel over batch — one batch sample (and its weight slabs)
per NeuronCore, 8 cores, no cross-core communication.

Device-side strategy: activations are kept feature-major ([feature, seq],
feature on partitions) so every layer is a plain accumulated matmul with the
(pre-transposed) weights as the stationary operand and the activations as the
moving operand — no on-chip transposes. The final layer flips orientation
(stationary = activation tile, moving = W2^T) so the output lands seq-major
and can be DMA'd out contiguously. Inputs are pre-transposed AND pre-cast to
bf16 on the host: bf16 streams at the PE's full 1 row/cycle (216 ns measured
per 128x128x512 matmul, the warm roofline) and halves all input DMA traffic.
fp8 was evaluated and rejected: DoubleRow e4m3 measures exactly 2x bf16 on
this hw (218.8 ns for a 2-k-tile matmul), but uncompensated e4m3 gives
3.7e-2..6.9e-2 end-to-end rel err (gate 2e-2) and the 3-pass hi/lo
compensation that fixes it costs 1.5x bf16 — a net loss. So the bf16 PE
roofline (~218.5 us for 1024 matmuls) is the floor and everything else is
startup/tail/gap engineering:

- Every input tensor is packed on the host so each DMA is a dense
  per-partition-contiguous 2D block ([128, F] with F contiguous): q is
  chunk-major [NCH, 128, K0*SC], weights are [128, K*out] k-major slabs.
  This minimizes HWDGE descriptor count (a q-chunk issue drops from
  ~1.5 us to ~0.6 us of sync-engine time), so the sync-queue issue of the
  startup loads (w0a q0a w0b q0b b0 q1 | w1 b1 | w2 b2, one FIFO ring in
  consumption order) completes fast enough that the ring never starves.
- Chunk 0's L0 is emitted k-split: each m-group's accumulation is two
  k-pair passes with the PSUM group left open in between, interleaved
  over 4 m-groups, and q0 arrives as two k-half DMAs. Pass A (k=0,1)
  then waits only on w0a + q0[k<2] (1.0 MiB) instead of the full
  w0+q0 (2 MiB), starting the real stream ~1.5 us earlier. All matmuls
  stay full-width N=512: variants using N=128 strips or narrow warmups
  left the whole run at the mid p-state (292 us vs 241 — the clock
  governor keys on sustained full-width array activity).
- The PE clock (DVFS) ramps only while the PE is busy, so dummy warmup
  matmuls on garbage data spin it up while the startup DMAs land. Warmup
  is sized to end right as pass A's data arrives: oversizing it delays
  the real stream; a short idle before the first real matmul is harmless
  (no clock decay observed at ~1.3 us idle).
- Warmup operand memsets run on GpSimd (idle at preamble end) so the PE
  starts ~0.9 us earlier than with DVE memsets.
- PSUM banks 4/2/2 (L0/L1/L2): consecutive L0(c)+L0(c+1) phases of the
  software pipeline emit 16 back-to-back 4-matmul groups (864 ns each)
  whose consumer (scalar relu, ~700 ns) barely keeps up; with only 3 L0
  banks the bank recycle stalls the PE ~0.8-1.2 us per run. L1/L2 groups
  are 8 matmuls (1728 ns) — 2 banks each drain comfortably.
- The output is stored bf16 (halves the output DMA and the strictly-serial
  tail after the last matmul) and widened to fp32 on the host. The final
  seq-tile's PSUM->add->DMA chain is quartered across both HWDGE rings.
  Accumulation stays fp32 in PSUM; measured end-to-end relative error is
  4.7e-3 against the fp32 reference, vs the 2e-2 gate.

Software pipeline: emit L0 of chunk c+1 ahead of L1/L2 of chunk c, so the
matmul stream never depends on a DMA issued less than a full chunk earlier,
and the 2 MiB w1 slab has landed before L1(c0) needs it.

Known structural costs (measured, not recoverable at this API level):
~6.1 us framework engine preamble before any user instruction; ~2.5 us
all-engine drain at the end; ~4.3 us of PE instruction-queue refill
bubbles (every 50th LDWEIGHTS stalls ~216 ns — critical_dep attribution;
period is locked to instruction count, and bass emits an LDWEIGHTS per
matmul unconditionally, even for a repeated stationary operand, which a
microbenchmark showed costs nothing extra anyway); and the 221.2 us PE
stream itself (1024 x 512-cycle matmuls at the 2.37 GHz steady clock).
Measured min-of-N ~240.1-241.3 us depending on device thermal state.
"""

import numpy as np

import ml_dtypes

import concourse.bass as bass
import concourse.tile as tile
from concourse import bacc, mybir
from concourse.bass_utils import run_bass_kernel_spmd

B, S, DIN, DH, DOUT = 8, 4096, 512, 1024, 512
SC = 512  # seq chunk processed per pipeline iteration

BF16 = mybir.dt.bfloat16
F32 = mybir.dt.float32

K0 = DIN // 128   # 4  k-tiles, layer 0
K1 = DH // 128    # 8  k-tiles, layers 1/2
M0 = DH // 128    # 8  m-tiles (feature tiles of h0/h1)
MT = SC // 128    # 4  seq m-tiles per chunk, layer 2
NCH = S // SC     # 8  chunks

N_WARM = 10


def build_nc():
    nc = bacc.Bacc("TRN2")
    # Host-packed dense layouts: every DMA below reads a per-partition
    # contiguous [128, F] block (minimal descriptors, fast issue).
    qd = nc.dram_tensor("qd", (NCH, 128, K0 * SC), BF16, kind="ExternalInput")
    w0d = nc.dram_tensor("w0d", (128, K0 * DH), BF16, kind="ExternalInput")
    w1d = nc.dram_tensor("w1d", (128, K1 * DH), BF16, kind="ExternalInput")
    w2d = nc.dram_tensor("w2d", (128, K1 * DOUT), BF16, kind="ExternalInput")
    # b0/b1 host-pre-transposed to [128, DH//128] (partition-major)
    b0 = nc.dram_tensor("b0", (128, DH // 128), F32, kind="ExternalInput")
    b1 = nc.dram_tensor("b1", (128, DH // 128), F32, kind="ExternalInput")
    b2 = nc.dram_tensor("b2", (DOUT,), F32, kind="ExternalInput")
    out = nc.dram_tensor("out", (S, DOUT), BF16, kind="ExternalOutput")

    Relu = mybir.ActivationFunctionType.Relu

    with tile.TileContext(nc) as tc:
        with (
            tc.tile_pool(name="weights", bufs=1) as wpool,
            tc.tile_pool(name="biases", bufs=1) as bpool,
            tc.tile_pool(name="acts", bufs=2) as apool,
            tc.tile_pool(name="qin", bufs=2) as qpool,
            tc.tile_pool(name="outp", bufs=4) as opool,
            tc.tile_pool(name="psum0", bufs=4, space="PSUM") as ppool0,
            tc.tile_pool(name="psum1", bufs=2, space="PSUM") as ppool1,
            tc.tile_pool(name="psum2", bufs=2, space="PSUM") as ppool2,
        ):
            # Pre-warm the PE clock with dummy matmuls on garbage data
            # while the startup DMAs land: the real matmul stream then
            # starts near 2.4 GHz. Full-width N=512 matmuls are required —
            # a variant with 40 tiny N=128 warmups left the whole run at
            # the mid p-state (292 us vs 241), so the governor appears to
            # key on sustained full-width array activity. GpSimd memsets:
            # it is the first engine free after the preamble, so the PE
            # starts ~0.9 us earlier than with DVE memsets.
            g_lhs = apool.tile([128, 128], BF16, tag="warm_lhs")
            g_rhs = apool.tile([128, SC], BF16, tag="warm_rhs")
            nc.gpsimd.memset(g_lhs, 0.0)
            nc.gpsimd.memset(g_rhs, 0.0)
            warm_ps = ppool0.tile([128, SC], F32, tag="ps0")
            for i in range(N_WARM):
                nc.tensor.matmul(
                    warm_ps, lhsT=g_lhs, rhs=g_rhs,
                    start=(i == 0), stop=(i == N_WARM - 1),
                )

            # Startup loads: ALL on the sync HWDGE ring, in the order the
            # PE consumes them (w0 q0-quarters b0 q1 | w1 b1 | w2 b2). One
            # queue streaming large DMAs sustains ~341-365 GB/s, while
            # splitting the same bytes across parallel rings drains slower
            # aggregate — and FIFO order on one ring guarantees w0/q0 are
            # never slowed by the later, slack-rich w1/w2 transfers.
            w0a_sb = wpool.tile([128, K0 // 2, DH], BF16, tag="w0a")
            w0b_sb = wpool.tile([128, K0 // 2, DH], BF16, tag="w0b")
            b0_sb = bpool.tile([128, M0], F32, tag="b0")
            b1_sb = bpool.tile([128, M0], F32, tag="b1")
            nc.sync.dma_start(out=w0a_sb, in_=w0d[:, 0:(K0 // 2) * DH])

            def w0_slice(k, m):
                t = w0a_sb if k < K0 // 2 else w0b_sb
                return t[:, k % (K0 // 2), m * 128:(m + 1) * 128]

            def load_q(c):
                t = qpool.tile([128, K0, SC], BF16, tag="q", name=f"q{c}")
                nc.sync.dma_start(out=t, in_=qd[c])
                return t

            # q0 in two k-half DMAs: pass A of the k-split below then waits
            # only on w0a + q0[k=0,1] (1.0 MiB) instead of the full q0.
            q0_sb = qpool.tile([128, K0, SC], BF16, tag="q", name="q0")
            nc.sync.dma_start(
                out=q0_sb[:, 0:K0 // 2, :], in_=qd[0][:, 0:(K0 // 2) * SC]
            )
            nc.sync.dma_start(out=w0b_sb, in_=w0d[:, (K0 // 2) * DH:K0 * DH])
            nc.sync.dma_start(
                out=q0_sb[:, K0 // 2:K0, :], in_=qd[0][:, (K0 // 2) * SC:K0 * SC]
            )
            nc.sync.dma_start(out=b0_sb, in_=b0[:, :])
            q1_sb = load_q(1)

            w1_sb = wpool.tile([128, K1, DH], BF16, tag="w1")
            nc.sync.dma_start(out=w1_sb, in_=w1d[:, :])
            nc.sync.dma_start(out=b1_sb, in_=b1[:, :])

            def w1_slice(k, m):
                return w1_sb[:, k, m * 128:(m + 1) * 128]

            w2_sb = wpool.tile([128, K1, DOUT], BF16, tag="w2")
            nc.sync.dma_start(out=w2_sb, in_=w2d[:, :])
            b2_sb = bpool.tile([128, DOUT], F32, tag="b2")
            b2_ap = b2[:]
            b2_bcast = bass.AP(
                tensor=b2_ap.tensor,
                offset=b2_ap.offset,
                ap=[[0, 128]] + [list(d) for d in b2_ap.ap],
            )
            nc.sync.dma_start(out=b2_sb, in_=b2_bcast)

            def layer0(c, q_sb):
                h0_sb = []
                for m in range(M0):
                    ps = ppool0.tile([128, SC], F32, tag="ps0", name=f"ps0_{c}_{m}")
                    for k in range(K0):
                        nc.tensor.matmul(
                            ps,
                            lhsT=w0_slice(k, m),
                            rhs=q_sb[:, k, :],
                            start=(k == 0),
                            stop=(k == K0 - 1),
                        )
                    h = apool.tile([128, SC], BF16, tag=f"h0_{m}", name=f"h0_{c}_{m}")
                    nc.scalar.activation(h, ps, Relu, bias=b0_sb[:, m:m + 1])
                    h0_sb.append(h)
                return h0_sb

            def layer0_c0(q_sb):
                # First chunk, k-split: each m-group's accumulation is
                # emitted as two k-pair passes with the PSUM group left
                # open in between, interleaved over 4 m-groups (= the 4
                # L0 banks). Pass A (k=0,1) needs only w0a+q0 (1.5 MiB,
                # ready ~12.3 us); w0b lands (~14 us) while pass A runs.
                # Full-width N=512 matmuls throughout (narrow warmups or
                # strips leave the clock governor at the mid p-state).
                h0_sb = [None] * M0
                for half in range(2):
                    ms = list(range(half * 4, half * 4 + 4))
                    pss = {}
                    for phase in range(2):
                        for m in ms:
                            if phase == 0:
                                pss[m] = ppool0.tile(
                                    [128, SC], F32, tag="ps0", name=f"ps0_0_{m}"
                                )
                            ps = pss[m]
                            for k in (phase * 2, phase * 2 + 1):
                                nc.tensor.matmul(
                                    ps,
                                    lhsT=w0_slice(k, m),
                                    rhs=q_sb[:, k, :],
                                    start=(k == 0),
                                    stop=(k == K0 - 1),
                                    skip_group_check=True,
                                )
                            if phase == 1:
                                h = apool.tile(
                                    [128, SC], BF16, tag=f"h0_{m}",
                                    name=f"h0_0_{m}",
                                )
                                nc.scalar.activation(
                                    h, ps, Relu, bias=b0_sb[:, m:m + 1]
                                )
                                h0_sb[m] = h
                return h0_sb

            def layer1(c, h0_sb):
                h1_sb = []
                for m in range(M0):
                    ps = ppool1.tile([128, SC], F32, tag="ps1", name=f"ps1_{c}_{m}")
                    for k in range(K1):
                        nc.tensor.matmul(
                            ps,
                            lhsT=w1_slice(k, m),
                            rhs=h0_sb[k],
                            start=(k == 0),
                            stop=(k == K1 - 1),
                        )
                    h = apool.tile([128, SC], BF16, tag=f"h1_{m}", name=f"h1_{c}_{m}")
                    nc.scalar.activation(h, ps, Relu, bias=b1_sb[:, m:m + 1])
                    h1_sb.append(h)
                return h1_sb

            def layer2(c, h1_sb):
                s0 = c * SC
                last = c == NCH - 1
                for mt in range(MT):
                    ps = ppool2.tile([128, DOUT], F32, tag="ps2", name=f"ps2_{c}_{mt}")
                    for k in range(K1):
                        nc.tensor.matmul(
                            ps,
                            lhsT=h1_sb[k][:, mt * 128:(mt + 1) * 128],
                            rhs=w2_sb[:, k, :],
                            start=(k == 0),
                            stop=(k == K1 - 1),
                        )
                    ot = opool.tile([128, DOUT], BF16, tag="ot", name=f"ot_{c}_{mt}")
                    r0 = s0 + mt * 128
                    if last and mt == MT - 1:
                        # Tail trim: quarter the strictly-serial
                        # PSUM->add->DMA chain after the very last matmul,
                        # alternating the two HWDGE rings so issue overlaps
                        # (halves were tried: 240.9 us min vs 240.3 — the
                        # longer first add outweighs the parallel issues).
                        Q = DOUT // 4
                        for i in range(4):
                            sl = slice(i * Q, (i + 1) * Q)
                            nc.vector.tensor_add(ot[:, sl], ps[:, sl], b2_sb[:, sl])
                            eng = nc.scalar if i % 2 == 0 else nc.sync
                            eng.dma_start(out=out[r0:r0 + 128, sl], in_=ot[:, sl])
                    else:
                        nc.vector.tensor_add(ot, ps, b2_sb)
                        eng = nc.scalar if mt % 2 == 0 else nc.sync
                        eng.dma_start(out=out[r0:r0 + 128, :], in_=ot)

            # Software pipeline: emit L0 of chunk c+1 ahead of L1/L2 of
            # chunk c, so the matmul stream never depends on a DMA issued
            # less than a full chunk earlier.
            h0_cur = layer0_c0(q0_sb)
            for c in range(NCH):
                h0_next = None
                if c + 1 < NCH:
                    h0_next = layer0(c + 1, q1_sb if c == 0 else load_q(c + 1))
                layer2(c, layer1(c, h0_cur))
                h0_cur = h0_next
    nc.finalize()
    return nc


_NC = None


def _get_nc():
    global _NC
    if _NC is None:
        _NC = build_nc()
    return _NC


def make_in_maps(inputs):
    bf16 = ml_dtypes.bfloat16
    q, W0, b0, W1, b1, W2, b2 = (
        inputs["query"], inputs["W0"], inputs["b0"], inputs["W1"],
        inputs["b1"], inputs["W2"], inputs["b2"],
    )
    in_maps = []
    for b in range(B):
        # qd[c, p, k*SC + s] = q[b, c*SC+s, k*128+p]
        qT = np.asarray(q[b]).T.astype(bf16)            # [DIN, S]
        qd = (
            qT.reshape(K0, 128, NCH, SC)
            .transpose(2, 1, 0, 3)
            .reshape(NCH, 128, K0 * SC)
        )

        # wXd[p, k*out + j] = WX[b].T[(k*128+p), j]
        w0t = np.asarray(W0[b]).T.astype(bf16)          # [DIN, DH]
        w0d = w0t.reshape(K0, 128, DH).transpose(1, 0, 2).reshape(128, K0 * DH)
        w1t = np.asarray(W1[b]).T.astype(bf16)          # [DH, DH]
        w1d = w1t.reshape(K1, 128, DH).transpose(1, 0, 2).reshape(128, K1 * DH)
        w2t = np.asarray(W2[b]).T.astype(bf16)          # [DH, DOUT]
        w2d = w2t.reshape(K1, 128, DOUT).transpose(1, 0, 2).reshape(128, K1 * DOUT)
        in_maps.append({
            "qd": np.ascontiguousarray(qd),
            "w0d": np.ascontiguousarray(w0d),
            "w1d": np.ascontiguousarray(w1d),
            "w2d": np.ascontiguousarray(w2d),
            "b0": np.ascontiguousarray(
                np.asarray(b0[b], dtype=np.float32).reshape(DH // 128, 128).T
            ),
            "b1": np.ascontiguousarray(
                np.asarray(b1[b], dtype=np.float32).reshape(DH // 128, 128).T
            ),
            "b2": np.asarray(b2[b], dtype=np.float32),
        })
    return in_maps


def run(inputs, trace=False):
    nc = _get_nc()
    in_maps = make_in_maps(inputs)
    res = run_bass_kernel_spmd(nc, in_maps, core_ids=list(range(B)), trace=trace)
    out = np.stack(
        [np.asarray(r["out"]).astype(np.float32) for r in res.results]
    )
    return out, res


def kernel(**inputs) -> np.ndarray:
    out, _ = run(inputs, trace=False)
    return out


# revision 28
# speedup vs baseline: 1.0143x; 1.0073x over previous
"""Trainium2 Bass kernel for nn_LongTermMemoryMLP.

Per-batch-weight 3-layer MLP:
    h0 = relu(q @ W0^T + b0); h1 = relu(h0 @ W1^T + b1); out = h1 @ W2^T + b2
with q: [B,S,DIN], W0: [B,DH,DIN], W1: [B,DH,DH], W2: [B,DOUT,DH], B=8.

Sharding: data-parallel over batch — one batch sample (and its weight slabs)
per NeuronCore, 8 cores, no cross-core communication.

Device-side strategy: activations are kept feature-major ([feature, seq],
feature on partitions) so every layer is a plain accumulated matmul with the
(pre-transposed) weights as the stationary operand and the activations as the
moving operand — no on-chip transposes. The final layer flips orientation
(stationary = activation tile, moving = W2^T) so the output lands seq-major
and can be DMA'd out contiguously. Inputs are pre-transposed AND pre-cast to
bf16 on the host: bf16 streams at the PE's full 1 row/cycle (216 ns measured
per 128x128x512 matmul, the warm roofline) and halves all input DMA traffic.
fp8 was evaluated and rejected: DoubleRow e4m3 measures exactly 2x bf16 on
this hw (218.8 ns for a 2-k-tile matmul), but uncompensated e4m3 gives
3.7e-2..6.9e-2 end-to-end rel err (gate 2e-2) and the 3-pass hi/lo
compensation that fixes it costs 1.5x bf16 — a net loss. So the bf16 PE
roofline (~218.5 us for 1024 matmuls) is the floor and everything else is
startup/tail/gap engineering:

- Every input tensor is packed on the host so each DMA is a dense
  per-partition-contiguous 2D block ([128, F] with F contiguous): q is
  chunk-major [NCH, 128, K0*SC], weights are [128, K*out] k-major slabs.
  This minimizes HWDGE descriptor count (a q-chunk issue drops from
  ~1.5 us to ~0.6 us of sync-engine time), so the sync-queue issue of the
  startup loads (w0a q0a w0b q0b b0 q1 | w1 b1 | w2 b2, one FIFO ring in
  consumption order) completes fast enough that the ring never starves.
- Chunk 0's L0 is emitted k-split: each m-group's accumulation is two
  k-pair passes with the PSUM group left open in between, interleaved
  over 4 m-groups, and q0 arrives as two k-half DMAs. Pass A (k=0,1)
  then waits only on w0a + q0[k<2] (1.0 MiB) instead of the full
  w0+q0 (2 MiB), starting the real stream ~1.5 us earlier. All matmuls
  stay full-width N=512: variants using N=128 strips or narrow warmups
  left the whole run at the mid p-state (292 us vs 241 — the clock
  governor keys on sustained full-width array activity).
- The PE clock (DVFS) ramps only while the PE is busy, so dummy warmup
  matmuls on garbage data spin it up while the startup DMAs land. Warmup
  is sized to end right as pass A's data arrives: oversizing it delays
  the real stream; a short idle before the first real matmul is harmless
  (no clock decay observed at ~1.3 us idle).
- Warmup operand memsets run on GpSimd (idle at preamble end) so the PE
  starts ~0.9 us earlier than with DVE memsets.
- PSUM banks 4/2/2 (L0/L1/L2): consecutive L0(c)+L0(c+1) phases of the
  software pipeline emit 16 back-to-back 4-matmul groups (864 ns each)
  whose consumer (scalar relu, ~700 ns) barely keeps up; with only 3 L0
  banks the bank recycle stalls the PE ~0.8-1.2 us per run. L1/L2 groups
  are 8 matmuls (1728 ns) — 2 banks each drain comfortably.
- The output is stored bf16 (halves the output DMA and the strictly-serial
  tail after the last matmul) and widened to fp32 on the host. The final
  seq-tile's PSUM->add->DMA chain is quartered across both HWDGE rings.
  Accumulation stays fp32 in PSUM; measured end-to-end relative error is
  4.7e-3 against the fp32 reference, vs the 2e-2 gate.

Software pipeline: emit L0 of chunk c+1 ahead of L1/L2 of chunk c, so the
matmul stream never depends on a DMA issued less than a full chunk earlier,
and the 2 MiB w1 slab has landed before L1(c0) needs it.

Known structural costs (measured, not recoverable at this API level):
~6.1 us framework engine preamble before any user instruction; ~2.5 us
all-engine drain at the end; ~4.3 us of PE instruction-queue refill
bubbles (every 50th LDWEIGHTS stalls ~216 ns — critical_dep attribution;
period is locked to instruction count, and bass emits an LDWEIGHTS per
matmul unconditionally, even for a repeated stationary operand, which a
microbenchmark showed costs nothing extra anyway); and the 221.2 us PE
stream itself (1024 x 512-cycle matmuls at the 2.37 GHz steady clock).
Measured min-of-N ~240.1-241.3 us depending on device thermal state.
"""

import numpy as np

import ml_dtypes

import concourse.bass as bass
import concourse.tile as tile
from concourse import bacc, mybir
from concourse.bass_utils import run_bass_kernel_spmd

B, S, DIN, DH, DOUT = 8, 4096, 512, 1024, 512
SC = 512  # seq chunk processed per pipeline iteration

BF16 = mybir.dt.bfloat16
F32 = mybir.dt.float32

K0 = DIN // 128   # 4  k-tiles, layer 0
K1 = DH // 128    # 8  k-tiles, layers 1/2
M0 = DH // 128    # 8  m-tiles (feature tiles of h0/h1)
MT = SC // 128    # 4  seq m-tiles per chunk, layer 2
NCH = S // SC     # 8  chunks

N_WARM = 10


def build_nc():
    nc = bacc.Bacc("TRN2")
    # Host-packed dense layouts: every DMA below reads a per-partition
    # contiguous [128, F] block (minimal descriptors, fast issue).
    qd = nc.dram_tensor("qd", (NCH, 128, K0 * SC), BF16, kind="ExternalInput")
    # w0 as four (k-half x m-half) quarters, each a dense [128, 2, 512]
    # slab: quarter j = mhalf*2 + khalf. Interleaved with the q0 k-halves
    # in the startup FIFO so every k-split pass of chunk 0 has its data
    # before the PE reaches it (zero startup stalls).
    w0q = nc.dram_tensor("w0q", (4, 128, 2 * (DH // 2)), BF16, kind="ExternalInput")
    w1d = nc.dram_tensor("w1d", (128, K1 * DH), BF16, kind="ExternalInput")
    w2d = nc.dram_tensor("w2d", (128, K1 * DOUT), BF16, kind="ExternalInput")
    # b0/b1 host-pre-transposed to [128, DH//128] (partition-major)
    b0 = nc.dram_tensor("b0", (128, DH // 128), F32, kind="ExternalInput")
    b1 = nc.dram_tensor("b1", (128, DH // 128), F32, kind="ExternalInput")
    b2 = nc.dram_tensor("b2", (DOUT,), F32, kind="ExternalInput")
    out = nc.dram_tensor("out", (S, DOUT), BF16, kind="ExternalOutput")

    Relu = mybir.ActivationFunctionType.Relu

    with tile.TileContext(nc) as tc:
        with (
            tc.tile_pool(name="weights", bufs=1) as wpool,
            tc.tile_pool(name="biases", bufs=1) as bpool,
            tc.tile_pool(name="acts", bufs=2) as apool,
            tc.tile_pool(name="qin", bufs=2) as qpool,
            tc.tile_pool(name="outp", bufs=4) as opool,
            tc.tile_pool(name="psum0", bufs=4, space="PSUM") as ppool0,
            tc.tile_pool(name="psum1", bufs=2, space="PSUM") as ppool1,
            tc.tile_pool(name="psum2", bufs=2, space="PSUM") as ppool2,
        ):
            # Pre-warm the PE clock with dummy matmuls on garbage data
            # while the startup DMAs land: the real matmul stream then
            # starts near 2.4 GHz. Full-width N=512 matmuls are required —
            # a variant with 40 tiny N=128 warmups left the whole run at
            # the mid p-state (292 us vs 241), so the governor appears to
            # key on sustained full-width array activity. GpSimd memsets:
            # it is the first engine free after the preamble, so the PE
            # starts ~0.9 us earlier than with DVE memsets.
            g_lhs = apool.tile([128, 128], BF16, tag="warm_lhs")
            g_rhs = apool.tile([128, SC], BF16, tag="warm_rhs")
            nc.gpsimd.memset(g_lhs, 0.0)
            nc.gpsimd.memset(g_rhs, 0.0)
            warm_ps = ppool0.tile([128, SC], F32, tag="ps0")
            for i in range(N_WARM):
                nc.tensor.matmul(
                    warm_ps, lhsT=g_lhs, rhs=g_rhs,
                    start=(i == 0), stop=(i == N_WARM - 1),
                )

            # Startup loads: ALL on the sync HWDGE ring, in the order the
            # PE consumes them (w0aa q0k01 w0ba b0 q0k23 w0ab w0bb q1 |
            # w1 b1 | w2 b2). One queue streaming large DMAs sustains
            # ~341-365 GB/s, while splitting the same bytes across parallel
            # rings drains slower aggregate — and FIFO order on one ring
            # guarantees chunk-0 data is never slowed by the later,
            # slack-rich w1/w2 transfers. With this order, pass A(m0-3)
            # needs only 0.75 MiB (~11.3 us) and every later pass's data
            # arrives before the PE reaches it.
            w0_sb = [[None, None], [None, None]]  # [khalf][mhalf]
            for kh in range(2):
                for mh in range(2):
                    w0_sb[kh][mh] = wpool.tile(
                        [128, 2, DH // 2], BF16, tag=f"w0_{kh}_{mh}",
                        name=f"w0_{kh}_{mh}",
                    )
            b0_sb = bpool.tile([128, M0], F32, tag="b0")
            b1_sb = bpool.tile([128, M0], F32, tag="b1")

            def w0_slice(k, m):
                t = w0_sb[k // 2][m // 4]
                return t[:, k % 2, (m % 4) * 128:(m % 4 + 1) * 128]

            def load_q(c):
                t = qpool.tile([128, K0, SC], BF16, tag="q", name=f"q{c}")
                nc.sync.dma_start(out=t, in_=qd[c])
                return t

            q0_sb = qpool.tile([128, K0, SC], BF16, tag="q", name="q0")
            nc.sync.dma_start(out=w0_sb[0][0], in_=w0q[0])
            nc.sync.dma_start(
                out=q0_sb[:, 0:K0 // 2, :], in_=qd[0][:, 0:(K0 // 2) * SC]
            )
            nc.sync.dma_start(out=w0_sb[1][0], in_=w0q[1])
            nc.sync.dma_start(out=b0_sb, in_=b0[:, :])
            nc.sync.dma_start(
                out=q0_sb[:, K0 // 2:K0, :], in_=qd[0][:, (K0 // 2) * SC:K0 * SC]
            )
            nc.sync.dma_start(out=w0_sb[0][1], in_=w0q[2])
            nc.sync.dma_start(out=w0_sb[1][1], in_=w0q[3])
            q1_sb = load_q(1)

            w1_sb = wpool.tile([128, K1, DH], BF16, tag="w1")
            nc.sync.dma_start(out=w1_sb, in_=w1d[:, :])
            nc.sync.dma_start(out=b1_sb, in_=b1[:, :])

            def w1_slice(k, m):
                return w1_sb[:, k, m * 128:(m + 1) * 128]

            w2_sb = wpool.tile([128, K1, DOUT], BF16, tag="w2")
            nc.sync.dma_start(out=w2_sb, in_=w2d[:, :])
            b2_sb = bpool.tile([128, DOUT], F32, tag="b2")
            b2_ap = b2[:]
            b2_bcast = bass.AP(
                tensor=b2_ap.tensor,
                offset=b2_ap.offset,
                ap=[[0, 128]] + [list(d) for d in b2_ap.ap],
            )
            nc.sync.dma_start(out=b2_sb, in_=b2_bcast)

            def layer0(c, q_sb):
                h0_sb = []
                for m in range(M0):
                    ps = ppool0.tile([128, SC], F32, tag="ps0", name=f"ps0_{c}_{m}")
                    for k in range(K0):
                        nc.tensor.matmul(
                            ps,
                            lhsT=w0_slice(k, m),
                            rhs=q_sb[:, k, :],
                            start=(k == 0),
                            stop=(k == K0 - 1),
                        )
                    h = apool.tile([128, SC], BF16, tag=f"h0_{m}", name=f"h0_{c}_{m}")
                    nc.scalar.activation(h, ps, Relu, bias=b0_sb[:, m:m + 1])
                    h0_sb.append(h)
                return h0_sb

            def layer0_c0(q_sb):
                # First chunk, k-split: each m-group's accumulation is
                # emitted as two k-pair passes with the PSUM group left
                # open in between, interleaved over 4 m-groups (= the 4
                # L0 banks). Pass A (k=0,1) needs only w0a+q0 (1.5 MiB,
                # ready ~12.3 us); w0b lands (~14 us) while pass A runs.
                # Full-width N=512 matmuls throughout (narrow warmups or
                # strips leave the clock governor at the mid p-state).
                h0_sb = [None] * M0
                for half in range(2):
                    ms = list(range(half * 4, half * 4 + 4))
                    pss = {}
                    for phase in range(2):
                        for m in ms:
                            if phase == 0:
                                pss[m] = ppool0.tile(
                                    [128, SC], F32, tag="ps0", name=f"ps0_0_{m}"
                                )
                            ps = pss[m]
                            for k in (phase * 2, phase * 2 + 1):
                                nc.tensor.matmul(
                                    ps,
                                    lhsT=w0_slice(k, m),
                                    rhs=q_sb[:, k, :],
                                    start=(k == 0),
                                    stop=(k == K0 - 1),
                                    skip_group_check=True,
                                )
                            if phase == 1:
                                h = apool.tile(
                                    [128, SC], BF16, tag=f"h0_{m}",
                                    name=f"h0_0_{m}",
                                )
                                nc.scalar.activation(
                                    h, ps, Relu, bias=b0_sb[:, m:m + 1]
                                )
                                h0_sb[m] = h
                return h0_sb

            def layer1(c, h0_sb):
                h1_sb = []
                for m in range(M0):
                    ps = ppool1.tile([128, SC], F32, tag="ps1", name=f"ps1_{c}_{m}")
                    for k in range(K1):
                        nc.tensor.matmul(
                            ps,
                            lhsT=w1_slice(k, m),
                            rhs=h0_sb[k],
                            start=(k == 0),
                            stop=(k == K1 - 1),
                        )
                    h = apool.tile([128, SC], BF16, tag=f"h1_{m}", name=f"h1_{c}_{m}")
                    nc.scalar.activation(h, ps, Relu, bias=b1_sb[:, m:m + 1])
                    h1_sb.append(h)
                return h1_sb

            def layer2(c, h1_sb):
                s0 = c * SC
                last = c == NCH - 1
                for mt in range(MT):
                    ps = ppool2.tile([128, DOUT], F32, tag="ps2", name=f"ps2_{c}_{mt}")
                    for k in range(K1):
                        nc.tensor.matmul(
                            ps,
                            lhsT=h1_sb[k][:, mt * 128:(mt + 1) * 128],
                            rhs=w2_sb[:, k, :],
                            start=(k == 0),
                            stop=(k == K1 - 1),
                        )
                    ot = opool.tile([128, DOUT], BF16, tag="ot", name=f"ot_{c}_{mt}")
                    r0 = s0 + mt * 128
                    if last and mt == MT - 1:
                        # Tail trim: quarter the strictly-serial
                        # PSUM->add->DMA chain after the very last matmul,
                        # alternating the two HWDGE rings so issue overlaps
                        # (halves were tried: 240.9 us min vs 240.3 — the
                        # longer first add outweighs the parallel issues).
                        Q = DOUT // 4
                        for i in range(4):
                            sl = slice(i * Q, (i + 1) * Q)
                            nc.vector.tensor_add(ot[:, sl], ps[:, sl], b2_sb[:, sl])
                            eng = nc.scalar if i % 2 == 0 else nc.sync
                            eng.dma_start(out=out[r0:r0 + 128, sl], in_=ot[:, sl])
                    else:
                        nc.vector.tensor_add(ot, ps, b2_sb)
                        eng = nc.scalar if mt % 2 == 0 else nc.sync
                        eng.dma_start(out=out[r0:r0 + 128, :], in_=ot)

            # Software pipeline: emit L0 of chunk c+1 ahead of L1/L2 of
            # chunk c, so the matmul stream never depends on a DMA issued
            # less than a full chunk earlier.
            h0_cur = layer0_c0(q0_sb)
            for c in range(NCH):
                h0_next = None
                if c + 1 < NCH:
                    h0_next = layer0(c + 1, q1_sb if c == 0 else load_q(c + 1))
                layer2(c, layer1(c, h0_cur))
                h0_cur = h0_next
    nc.finalize()
    return nc


_NC = None


def _get_nc():
    global _NC
    if _NC is None:
        _NC = build_nc()
    return _NC


def make_in_maps(inputs):
    bf16 = ml_dtypes.bfloat16
    q, W0, b0, W1, b1, W2, b2 = (
        inputs["query"], inputs["W0"], inputs["b0"], inputs["W1"],
        inputs["b1"], inputs["W2"], inputs["b2"],
    )
    in_maps = []
    for b in range(B):
        # qd[c, p, k*SC + s] = q[b, c*SC+s, k*128+p]
        qT = np.asarray(q[b]).T.astype(bf16)            # [DIN, S]
        qd = (
            qT.reshape(K0, 128, NCH, SC)
            .transpose(2, 1, 0, 3)
            .reshape(NCH, 128, K0 * SC)
        )

        # wXd[p, k*out + j] = WX[b].T[(k*128+p), j]
        w0t = np.asarray(W0[b]).T.astype(bf16)          # [DIN, DH]
        # w0q[mh*2+kh, p, kk*512 + col] = w0t[(kh*2+kk)*128 + p, mh*512 + col]
        w0q = (
            w0t.reshape(2, 2, 128, 2, DH // 2)          # [kh, kk, p, mh, col]
            .transpose(3, 0, 2, 1, 4)                   # [mh, kh, p, kk, col]
            .reshape(4, 128, DH)
        )
        w1t = np.asarray(W1[b]).T.astype(bf16)          # [DH, DH]
        w1d = w1t.reshape(K1, 128, DH).transpose(1, 0, 2).reshape(128, K1 * DH)
        w2t = np.asarray(W2[b]).T.astype(bf16)          # [DH, DOUT]
        w2d = w2t.reshape(K1, 128, DOUT).transpose(1, 0, 2).reshape(128, K1 * DOUT)
        in_maps.append({
            "qd": np.ascontiguousarray(qd),
            "w0q": np.ascontiguousarray(w0q),
            "w1d": np.ascontiguousarray(w1d),
            "w2d": np.ascontiguousarray(w2d),
            "b0": np.ascontiguousarray(
                np.asarray(b0[b], dtype=np.float32).reshape(DH // 128, 128).T
            ),
            "b1": np.ascontiguousarray(
                np.asarray(b1[b], dtype=np.float32).reshape(DH // 128, 128).T
            ),
            "b2": np.asarray(b2[b], dtype=np.float32),
        })
    return in_maps


def run(inputs, trace=False):
    nc = _get_nc()
    in_maps = make_in_maps(inputs)
    res = run_bass_kernel_spmd(nc, in_maps, core_ids=list(range(B)), trace=trace)
    out = np.stack(
        [np.asarray(r["out"]).astype(np.float32) for r in res.results]
    )
    return out, res


def kernel(**inputs) -> np.ndarray:
    out, _ = run(inputs, trace=False)
    return out


# revision 29
# speedup vs baseline: 1.0146x; 1.0002x over previous
"""Trainium2 Bass kernel for nn_LongTermMemoryMLP.

Per-batch-weight 3-layer MLP:
    h0 = relu(q @ W0^T + b0); h1 = relu(h0 @ W1^T + b1); out = h1 @ W2^T + b2
with q: [B,S,DIN], W0: [B,DH,DIN], W1: [B,DH,DH], W2: [B,DOUT,DH], B=8.

Sharding: data-parallel over batch — one batch sample (and its weight slabs)
per NeuronCore, 8 cores, no cross-core communication.

Device-side strategy: activations are kept feature-major ([feature, seq],
feature on partitions) so every layer is a plain accumulated matmul with the
(pre-transposed) weights as the stationary operand and the activations as the
moving operand — no on-chip transposes. The final layer flips orientation
(stationary = activation tile, moving = W2^T) so the output lands seq-major
and can be DMA'd out contiguously. Inputs are pre-transposed AND pre-cast to
bf16 on the host: bf16 streams at the PE's full 1 row/cycle (216 ns measured
per 128x128x512 matmul, the warm roofline) and halves all input DMA traffic.
fp8 was evaluated and rejected: DoubleRow e4m3 measures exactly 2x bf16 on
this hw (218.8 ns for a 2-k-tile matmul), but uncompensated e4m3 gives
3.7e-2..6.9e-2 end-to-end rel err (gate 2e-2) and the 3-pass hi/lo
compensation that fixes it costs 1.5x bf16 — a net loss. So the bf16 PE
roofline (~218.5 us for 1024 matmuls) is the floor and everything else is
startup/tail/gap engineering:

- Every input tensor is packed on the host so each DMA is a dense
  per-partition-contiguous 2D block ([128, F] with F contiguous): q is
  chunk-major [NCH, 128, K0*SC], weights are [128, K*out] k-major slabs.
  This minimizes HWDGE descriptor count (a q-chunk issue drops from
  ~1.5 us to ~0.6 us of sync-engine time), so the sync-queue issue of the
  startup loads (w0a q0a w0b q0b b0 q1 | w1 b1 | w2 b2, one FIFO ring in
  consumption order) completes fast enough that the ring never starves.
- Chunk 0's L0 is emitted k-split: each m-group's accumulation is two
  k-pair passes with the PSUM group left open in between, interleaved
  over 4 m-groups, and q0 arrives as two k-half DMAs. Pass A (k=0,1)
  then waits only on w0a + q0[k<2] (1.0 MiB) instead of the full
  w0+q0 (2 MiB), starting the real stream ~1.5 us earlier. All matmuls
  stay full-width N=512: variants using N=128 strips or narrow warmups
  left the whole run at the mid p-state (292 us vs 241 — the clock
  governor keys on sustained full-width array activity).
- The PE clock (DVFS) ramps only while the PE is busy, so dummy warmup
  matmuls on garbage data spin it up while the startup DMAs land. Warmup
  is sized to end right as pass A's data arrives: oversizing it delays
  the real stream; a short idle before the first real matmul is harmless
  (no clock decay observed at ~1.3 us idle).
- Warmup operand memsets run on GpSimd (idle at preamble end) so the PE
  starts ~0.9 us earlier than with DVE memsets.
- PSUM banks 4/2/2 (L0/L1/L2): consecutive L0(c)+L0(c+1) phases of the
  software pipeline emit 16 back-to-back 4-matmul groups (864 ns each)
  whose consumer (scalar relu, ~700 ns) barely keeps up; with only 3 L0
  banks the bank recycle stalls the PE ~0.8-1.2 us per run. L1/L2 groups
  are 8 matmuls (1728 ns) — 2 banks each drain comfortably.
- The output is stored bf16 (halves the output DMA and the strictly-serial
  tail after the last matmul) and widened to fp32 on the host. The final
  seq-tile's PSUM->add->DMA chain is quartered across both HWDGE rings.
  Accumulation stays fp32 in PSUM; measured end-to-end relative error is
  4.7e-3 against the fp32 reference, vs the 2e-2 gate.

Software pipeline: emit L0 of chunk c+1 ahead of L1/L2 of chunk c, so the
matmul stream never depends on a DMA issued less than a full chunk earlier,
and the 2 MiB w1 slab has landed before L1(c0) needs it.

Known structural costs (measured, not recoverable at this API level):
~6.1 us framework engine preamble before any user instruction; ~2.5 us
all-engine drain at the end; ~4.3 us of PE instruction-queue refill
bubbles (every 50th LDWEIGHTS stalls ~216 ns — critical_dep attribution;
period is locked to instruction count, and bass emits an LDWEIGHTS per
matmul unconditionally, even for a repeated stationary operand, which a
microbenchmark showed costs nothing extra anyway); and the 221.2 us PE
stream itself (1024 x 512-cycle matmuls at the 2.37 GHz steady clock).
Measured min-of-N ~240.1-241.3 us depending on device thermal state.
"""

import numpy as np

import ml_dtypes

import concourse.bass as bass
import concourse.tile as tile
from concourse import bacc, mybir
from concourse.bass_utils import run_bass_kernel_spmd

B, S, DIN, DH, DOUT = 8, 4096, 512, 1024, 512
SC = 512  # seq chunk processed per pipeline iteration

BF16 = mybir.dt.bfloat16
F32 = mybir.dt.float32

K0 = DIN // 128   # 4  k-tiles, layer 0
K1 = DH // 128    # 8  k-tiles, layers 1/2
M0 = DH // 128    # 8  m-tiles (feature tiles of h0/h1)
MT = SC // 128    # 4  seq m-tiles per chunk, layer 2
NCH = S // SC     # 8  chunks

N_WARM = 8


def build_nc():
    nc = bacc.Bacc("TRN2")
    # Host-packed dense layouts: every DMA below reads a per-partition
    # contiguous [128, F] block (minimal descriptors, fast issue).
    qd = nc.dram_tensor("qd", (NCH, 128, K0 * SC), BF16, kind="ExternalInput")
    # w0 as four (k-half x m-half) quarters, each a dense [128, 2, 512]
    # slab: quarter j = mhalf*2 + khalf. Interleaved with the q0 k-halves
    # in the startup FIFO so every k-split pass of chunk 0 has its data
    # before the PE reaches it (zero startup stalls).
    w0q = nc.dram_tensor("w0q", (4, 128, 2 * (DH // 2)), BF16, kind="ExternalInput")
    w1d = nc.dram_tensor("w1d", (128, K1 * DH), BF16, kind="ExternalInput")
    w2d = nc.dram_tensor("w2d", (128, K1 * DOUT), BF16, kind="ExternalInput")
    # b0/b1 host-pre-transposed to [128, DH//128] (partition-major)
    b0 = nc.dram_tensor("b0", (128, DH // 128), F32, kind="ExternalInput")
    b1 = nc.dram_tensor("b1", (128, DH // 128), F32, kind="ExternalInput")
    b2 = nc.dram_tensor("b2", (DOUT,), F32, kind="ExternalInput")
    out = nc.dram_tensor("out", (S, DOUT), BF16, kind="ExternalOutput")

    Relu = mybir.ActivationFunctionType.Relu

    with tile.TileContext(nc) as tc:
        with (
            tc.tile_pool(name="weights", bufs=1) as wpool,
            tc.tile_pool(name="biases", bufs=1) as bpool,
            tc.tile_pool(name="acts", bufs=2) as apool,
            tc.tile_pool(name="qin", bufs=2) as qpool,
            tc.tile_pool(name="outp", bufs=4) as opool,
            tc.tile_pool(name="psum0", bufs=4, space="PSUM") as ppool0,
            tc.tile_pool(name="psum1", bufs=2, space="PSUM") as ppool1,
            tc.tile_pool(name="psum2", bufs=2, space="PSUM") as ppool2,
        ):
            # Pre-warm the PE clock with dummy matmuls on garbage data
            # while the startup DMAs land: the real matmul stream then
            # starts near 2.4 GHz. Full-width N=512 matmuls are required —
            # a variant with 40 tiny N=128 warmups left the whole run at
            # the mid p-state (292 us vs 241), so the governor appears to
            # key on sustained full-width array activity. GpSimd memsets:
            # it is the first engine free after the preamble, so the PE
            # starts ~0.9 us earlier than with DVE memsets.
            g_lhs = apool.tile([128, 128], BF16, tag="warm_lhs")
            g_rhs = apool.tile([128, SC], BF16, tag="warm_rhs")
            nc.gpsimd.memset(g_lhs, 0.0)
            nc.gpsimd.memset(g_rhs, 0.0)
            warm_ps = ppool0.tile([128, SC], F32, tag="ps0")
            for i in range(N_WARM):
                nc.tensor.matmul(
                    warm_ps, lhsT=g_lhs, rhs=g_rhs,
                    start=(i == 0), stop=(i == N_WARM - 1),
                )

            # Startup loads: ALL on the sync HWDGE ring, in the order the
            # PE consumes them (w0aa q0k01 w0ba b0 q0k23 w0ab w0bb q1 |
            # w1 b1 | w2 b2). One queue streaming large DMAs sustains
            # ~341-365 GB/s, while splitting the same bytes across parallel
            # rings drains slower aggregate — and FIFO order on one ring
            # guarantees chunk-0 data is never slowed by the later,
            # slack-rich w1/w2 transfers. With this order, pass A(m0-3)
            # needs only 0.75 MiB (~11.3 us) and every later pass's data
            # arrives before the PE reaches it.
            w0_sb = [[None, None], [None, None]]  # [khalf][mhalf]
            for kh in range(2):
                for mh in range(2):
                    w0_sb[kh][mh] = wpool.tile(
                        [128, 2, DH // 2], BF16, tag=f"w0_{kh}_{mh}",
                        name=f"w0_{kh}_{mh}",
                    )
            b0_sb = bpool.tile([128, M0], F32, tag="b0")
            b1_sb = bpool.tile([128, M0], F32, tag="b1")

            def w0_slice(k, m):
                t = w0_sb[k // 2][m // 4]
                return t[:, k % 2, (m % 4) * 128:(m % 4 + 1) * 128]

            def load_q(c):
                t = qpool.tile([128, K0, SC], BF16, tag="q", name=f"q{c}")
                nc.sync.dma_start(out=t, in_=qd[c])
                return t

            q0_sb = qpool.tile([128, K0, SC], BF16, tag="q", name="q0")
            nc.sync.dma_start(out=w0_sb[0][0], in_=w0q[0])
            nc.sync.dma_start(
                out=q0_sb[:, 0:K0 // 2, :], in_=qd[0][:, 0:(K0 // 2) * SC]
            )
            nc.sync.dma_start(out=w0_sb[1][0], in_=w0q[1])
            nc.sync.dma_start(out=b0_sb, in_=b0[:, :])
            nc.sync.dma_start(
                out=q0_sb[:, K0 // 2:K0, :], in_=qd[0][:, (K0 // 2) * SC:K0 * SC]
            )
            nc.sync.dma_start(out=w0_sb[0][1], in_=w0q[2])
            nc.sync.dma_start(out=w0_sb[1][1], in_=w0q[3])
            q1_sb = load_q(1)

            w1_sb = wpool.tile([128, K1, DH], BF16, tag="w1")
            nc.sync.dma_start(out=w1_sb, in_=w1d[:, :])
            nc.sync.dma_start(out=b1_sb, in_=b1[:, :])

            def w1_slice(k, m):
                return w1_sb[:, k, m * 128:(m + 1) * 128]

            w2_sb = wpool.tile([128, K1, DOUT], BF16, tag="w2")
            nc.sync.dma_start(out=w2_sb, in_=w2d[:, :])
            b2_sb = bpool.tile([128, DOUT], F32, tag="b2")
            b2_ap = b2[:]
            b2_bcast = bass.AP(
                tensor=b2_ap.tensor,
                offset=b2_ap.offset,
                ap=[[0, 128]] + [list(d) for d in b2_ap.ap],
            )
            nc.sync.dma_start(out=b2_sb, in_=b2_bcast)

            def layer0(c, q_sb):
                h0_sb = []
                for m in range(M0):
                    ps = ppool0.tile([128, SC], F32, tag="ps0", name=f"ps0_{c}_{m}")
                    for k in range(K0):
                        nc.tensor.matmul(
                            ps,
                            lhsT=w0_slice(k, m),
                            rhs=q_sb[:, k, :],
                            start=(k == 0),
                            stop=(k == K0 - 1),
                        )
                    h = apool.tile([128, SC], BF16, tag=f"h0_{m}", name=f"h0_{c}_{m}")
                    nc.scalar.activation(h, ps, Relu, bias=b0_sb[:, m:m + 1])
                    h0_sb.append(h)
                return h0_sb

            def layer0_c0(q_sb):
                # First chunk, k-split: each m-group's accumulation is
                # emitted as two k-pair passes with the PSUM group left
                # open in between, interleaved over 4 m-groups (= the 4
                # L0 banks). Pass A (k=0,1) needs only w0a+q0 (1.5 MiB,
                # ready ~12.3 us); w0b lands (~14 us) while pass A runs.
                # Full-width N=512 matmuls throughout (narrow warmups or
                # strips leave the clock governor at the mid p-state).
                h0_sb = [None] * M0
                for half in range(2):
                    ms = list(range(half * 4, half * 4 + 4))
                    pss = {}
                    for phase in range(2):
                        for m in ms:
                            if phase == 0:
                                pss[m] = ppool0.tile(
                                    [128, SC], F32, tag="ps0", name=f"ps0_0_{m}"
                                )
                            ps = pss[m]
                            for k in (phase * 2, phase * 2 + 1):
                                nc.tensor.matmul(
                                    ps,
                                    lhsT=w0_slice(k, m),
                                    rhs=q_sb[:, k, :],
                                    start=(k == 0),
                                    stop=(k == K0 - 1),
                                    skip_group_check=True,
                                )
                            if phase == 1:
                                h = apool.tile(
                                    [128, SC], BF16, tag=f"h0_{m}",
                                    name=f"h0_0_{m}",
                                )
                                nc.scalar.activation(
                                    h, ps, Relu, bias=b0_sb[:, m:m + 1]
                                )
                                h0_sb[m] = h
                return h0_sb

            def layer1(c, h0_sb):
                h1_sb = []
                for m in range(M0):
                    ps = ppool1.tile([128, SC], F32, tag="ps1", name=f"ps1_{c}_{m}")
                    for k in range(K1):
                        nc.tensor.matmul(
                            ps,
                            lhsT=w1_slice(k, m),
                            rhs=h0_sb[k],
                            start=(k == 0),
                            stop=(k == K1 - 1),
                        )
                    h = apool.tile([128, SC], BF16, tag=f"h1_{m}", name=f"h1_{c}_{m}")
                    nc.scalar.activation(h, ps, Relu, bias=b1_sb[:, m:m + 1])
                    h1_sb.append(h)
                return h1_sb

            def layer2(c, h1_sb):
                s0 = c * SC
                last = c == NCH - 1
                for mt in range(MT):
                    ps = ppool2.tile([128, DOUT], F32, tag="ps2", name=f"ps2_{c}_{mt}")
                    for k in range(K1):
                        nc.tensor.matmul(
                            ps,
                            lhsT=h1_sb[k][:, mt * 128:(mt + 1) * 128],
                            rhs=w2_sb[:, k, :],
                            start=(k == 0),
                            stop=(k == K1 - 1),
                        )
                    ot = opool.tile([128, DOUT], BF16, tag="ot", name=f"ot_{c}_{mt}")
                    r0 = s0 + mt * 128
                    if last and mt == MT - 1:
                        # Tail trim: quarter the strictly-serial
                        # PSUM->add->DMA chain after the very last matmul,
                        # alternating the two HWDGE rings so issue overlaps
                        # (halves were tried: 240.9 us min vs 240.3 — the
                        # longer first add outweighs the parallel issues).
                        Q = DOUT // 4
                        for i in range(4):
                            sl = slice(i * Q, (i + 1) * Q)
                            nc.vector.tensor_add(ot[:, sl], ps[:, sl], b2_sb[:, sl])
                            eng = nc.scalar if i % 2 == 0 else nc.sync
                            eng.dma_start(out=out[r0:r0 + 128, sl], in_=ot[:, sl])
                    else:
                        nc.vector.tensor_add(ot, ps, b2_sb)
                        eng = nc.scalar if mt % 2 == 0 else nc.sync
                        eng.dma_start(out=out[r0:r0 + 128, :], in_=ot)

            # Software pipeline: emit L0 of chunk c+1 ahead of L1/L2 of
            # chunk c, so the matmul stream never depends on a DMA issued
            # less than a full chunk earlier.
            h0_cur = layer0_c0(q0_sb)
            for c in range(NCH):
                h0_next = None
                if c + 1 < NCH:
                    h0_next = layer0(c + 1, q1_sb if c == 0 else load_q(c + 1))
                layer2(c, layer1(c, h0_cur))
                h0_cur = h0_next
    nc.finalize()
    return nc


_NC = None


def _get_nc():
    global _NC
    if _NC is None:
        _NC = build_nc()
    return _NC


def make_in_maps(inputs):
    bf16 = ml_dtypes.bfloat16
    q, W0, b0, W1, b1, W2, b2 = (
        inputs["query"], inputs["W0"], inputs["b0"], inputs["W1"],
        inputs["b1"], inputs["W2"], inputs["b2"],
    )
    in_maps = []
    for b in range(B):
        # qd[c, p, k*SC + s] = q[b, c*SC+s, k*128+p]
        qT = np.asarray(q[b]).T.astype(bf16)            # [DIN, S]
        qd = (
            qT.reshape(K0, 128, NCH, SC)
            .transpose(2, 1, 0, 3)
            .reshape(NCH, 128, K0 * SC)
        )

        # wXd[p, k*out + j] = WX[b].T[(k*128+p), j]
        w0t = np.asarray(W0[b]).T.astype(bf16)          # [DIN, DH]
        # w0q[mh*2+kh, p, kk*512 + col] = w0t[(kh*2+kk)*128 + p, mh*512 + col]
        w0q = (
            w0t.reshape(2, 2, 128, 2, DH // 2)          # [kh, kk, p, mh, col]
            .transpose(3, 0, 2, 1, 4)                   # [mh, kh, p, kk, col]
            .reshape(4, 128, DH)
        )
        w1t = np.asarray(W1[b]).T.astype(bf16)          # [DH, DH]
        w1d = w1t.reshape(K1, 128, DH).transpose(1, 0, 2).reshape(128, K1 * DH)
        w2t = np.asarray(W2[b]).T.astype(bf16)          # [DH, DOUT]
        w2d = w2t.reshape(K1, 128, DOUT).transpose(1, 0, 2).reshape(128, K1 * DOUT)
        in_maps.append({
            "qd": np.ascontiguousarray(qd),
            "w0q": np.ascontiguousarray(w0q),
            "w1d": np.ascontiguousarray(w1d),
            "w2d": np.ascontiguousarray(w2d),
            "b0": np.ascontiguousarray(
                np.asarray(b0[b], dtype=np.float32).reshape(DH // 128, 128).T
            ),
            "b1": np.ascontiguousarray(
                np.asarray(b1[b], dtype=np.float32).reshape(DH // 128, 128).T
            ),
            "b2": np.asarray(b2[b], dtype=np.float32),
        })
    return in_maps


def run(inputs, trace=False):
    nc = _get_nc()
    in_maps = make_in_maps(inputs)
    res = run_bass_kernel_spmd(nc, in_maps, core_ids=list(range(B)), trace=trace)
    out = np.stack(
        [np.asarray(r["out"]).astype(np.float32) for r in res.results]
    )
    return out, res


def kernel(**inputs) -> np.ndarray:
    out, _ = run(inputs, trace=False)
    return out


# revision 31
# speedup vs baseline: 1.0165x; 1.0019x over previous
"""Trainium2 Bass kernel for nn_LongTermMemoryMLP.

Per-batch-weight 3-layer MLP:
    h0 = relu(q @ W0^T + b0); h1 = relu(h0 @ W1^T + b1); out = h1 @ W2^T + b2
with q: [B,S,DIN], W0: [B,DH,DIN], W1: [B,DH,DH], W2: [B,DOUT,DH], B=8.

Sharding: data-parallel over batch — one batch sample (and its weight slabs)
per NeuronCore, 8 cores, no cross-core communication.

Device-side strategy: activations are kept feature-major ([feature, seq],
feature on partitions) so every layer is a plain accumulated matmul with the
(pre-transposed) weights as the stationary operand and the activations as the
moving operand — no on-chip transposes. The final layer flips orientation
(stationary = activation tile, moving = W2^T) so the output lands seq-major
and can be DMA'd out contiguously. Inputs are pre-transposed AND pre-cast to
bf16 on the host: bf16 streams at the PE's full 1 row/cycle (216 ns measured
per 128x128x512 matmul, the warm roofline) and halves all input DMA traffic.
fp8 was evaluated and rejected: DoubleRow e4m3 measures exactly 2x bf16 on
this hw (218.8 ns for a 2-k-tile matmul), but uncompensated e4m3 gives
3.7e-2..6.9e-2 end-to-end rel err (gate 2e-2) and the 3-pass hi/lo
compensation that fixes it costs 1.5x bf16 — a net loss. So the bf16 PE
roofline (~218.5 us for 1024 matmuls) is the floor and everything else is
startup/tail/gap engineering:

- Every input tensor is packed on the host so each DMA is a dense
  per-partition-contiguous 2D block ([128, F] with F contiguous): q is
  chunk-major [NCH, 128, K0*SC], weights are [128, K*out] k-major slabs.
  This minimizes HWDGE descriptor count (a q-chunk issue drops from
  ~1.5 us to ~0.6 us of sync-engine time), so the sync-queue issue of the
  startup loads (one FIFO ring in consumption order) completes fast
  enough that the ring never starves.
- Chunk 0's L0 is emitted k-split: each m-group's accumulation is two
  k-pair passes with the PSUM group left open in between, interleaved
  over 4 m-groups. w0 is host-packed as four (k-half x m-half) quarters
  and q0 as two k-halves, issued interleaved (w0aa q0k01 w0ba b0 q0k23
  w0ab w0bb q1 | w1 b1 | w2 b2) so pass A(m0-3) needs only 0.75 MiB
  (first real matmul ~11.5-12.5 us vs ~14 us for a monolithic w0+q0) and
  every later pass's slab lands before the PE reaches it — the startup
  DMA stalls measured with coarser layouts (0.8-1.2 us) are zero here.
  All matmuls stay full-width N=512: variants using N=128 strips or
  narrow warmups left the whole run at the mid p-state (292 us vs 241 —
  the clock governor keys on sustained full-width array activity).
- The PE clock (DVFS) ramps only while the PE is busy, so dummy warmup
  matmuls on garbage data spin it up while the startup DMAs land. Warmup
  (N_WARM=8) is sized to end right as pass A's data arrives: oversizing
  it delays the real stream; a short idle before the first real matmul
  is harmless (no clock decay observed at ~1.3 us idle).
- Warmup operand memsets run on GpSimd (idle at preamble end) so the PE
  starts ~0.9 us earlier than with DVE memsets.
- PSUM banks 4/2/2 (L0/L1/L2): consecutive L0(c)+L0(c+1) phases of the
  software pipeline emit 16 back-to-back 4-matmul groups (864 ns each)
  whose consumer (scalar relu, ~700 ns) barely keeps up; with only 3 L0
  banks the bank recycle stalls the PE ~0.8-1.2 us per run. L1/L2 groups
  are 8 matmuls (1728 ns) — 2 banks each drain comfortably.
- The output is stored bf16 (halves the output DMA and the strictly-serial
  tail after the last matmul) and widened to fp32 on the host. The final
  seq-tile's PSUM->add->DMA chain is quartered across both HWDGE rings.
  Accumulation stays fp32 in PSUM; measured end-to-end relative error is
  4.7e-3 against the fp32 reference, vs the 2e-2 gate.

Software pipeline: emit L0 of chunk c+1 ahead of L1/L2 of chunk c, so the
matmul stream never depends on a DMA issued less than a full chunk earlier,
and the 2 MiB w1 slab has landed before L1(c0) needs it.

Known structural costs (measured, not recoverable at this API level):
~6.1 us framework engine preamble before any user instruction; ~2.5 us
all-engine drain at the end; ~4.3 us of PE instruction-queue refill
bubbles (every 50th LDWEIGHTS stalls ~216 ns — critical_dep attribution;
period is locked to instruction count, and bass emits an LDWEIGHTS per
matmul unconditionally, even for a repeated stationary operand, which a
microbenchmark showed costs nothing extra anyway); and the 221.2 us PE
stream itself (1024 x 512-cycle matmuls at the 2.37 GHz steady clock).
Measured min-of-N ~238.3-239.0 us depending on device thermal state.
"""

import numpy as np

import ml_dtypes

import concourse.bass as bass
import concourse.tile as tile
from concourse import bacc, mybir
from concourse.bass_utils import run_bass_kernel_spmd

B, S, DIN, DH, DOUT = 8, 4096, 512, 1024, 512
SC = 512  # seq chunk processed per pipeline iteration

BF16 = mybir.dt.bfloat16
F32 = mybir.dt.float32

K0 = DIN // 128   # 4  k-tiles, layer 0
K1 = DH // 128    # 8  k-tiles, layers 1/2
M0 = DH // 128    # 8  m-tiles (feature tiles of h0/h1)
MT = SC // 128    # 4  seq m-tiles per chunk, layer 2
NCH = S // SC     # 8  chunks

N_WARM = 8


def build_nc():
    nc = bacc.Bacc("TRN2")
    # Host-packed dense layouts: every DMA below reads a per-partition
    # contiguous [128, F] block (minimal descriptors, fast issue).
    qd = nc.dram_tensor("qd", (NCH, 128, K0 * SC), BF16, kind="ExternalInput")
    # w0 as four (k-half x m-half) quarters, each a dense [128, 2, 512]
    # slab: quarter j = mhalf*2 + khalf. Interleaved with the q0 k-halves
    # in the startup FIFO so every k-split pass of chunk 0 has its data
    # before the PE reaches it (zero startup stalls).
    w0q = nc.dram_tensor("w0q", (4, 128, 2 * (DH // 2)), BF16, kind="ExternalInput")
    w1d = nc.dram_tensor("w1d", (128, K1 * DH), BF16, kind="ExternalInput")
    w2d = nc.dram_tensor("w2d", (128, K1 * DOUT), BF16, kind="ExternalInput")
    # b0/b1 host-pre-transposed to [128, DH//128] (partition-major)
    b0 = nc.dram_tensor("b0", (128, DH // 128), F32, kind="ExternalInput")
    b1 = nc.dram_tensor("b1", (128, DH // 128), F32, kind="ExternalInput")
    b2 = nc.dram_tensor("b2", (DOUT,), F32, kind="ExternalInput")
    out = nc.dram_tensor("out", (S, DOUT), BF16, kind="ExternalOutput")

    Relu = mybir.ActivationFunctionType.Relu

    with tile.TileContext(nc) as tc:
        with (
            tc.tile_pool(name="weights", bufs=1) as wpool,
            tc.tile_pool(name="biases", bufs=1) as bpool,
            tc.tile_pool(name="acts", bufs=2) as apool,
            tc.tile_pool(name="qin", bufs=2) as qpool,
            tc.tile_pool(name="outp", bufs=4) as opool,
            tc.tile_pool(name="psum0", bufs=4, space="PSUM") as ppool0,
            tc.tile_pool(name="psum1", bufs=2, space="PSUM") as ppool1,
            tc.tile_pool(name="psum2", bufs=2, space="PSUM") as ppool2,
        ):
            # Pre-warm the PE clock with dummy matmuls on garbage data
            # while the startup DMAs land: the real matmul stream then
            # starts near 2.4 GHz. Full-width N=512 matmuls are required —
            # a variant with 40 tiny N=128 warmups left the whole run at
            # the mid p-state (292 us vs 241), so the governor appears to
            # key on sustained full-width array activity. GpSimd memsets:
            # it is the first engine free after the preamble, so the PE
            # starts ~0.9 us earlier than with DVE memsets.
            g_lhs = apool.tile([128, 128], BF16, tag="warm_lhs")
            g_rhs = apool.tile([128, SC], BF16, tag="warm_rhs")
            nc.gpsimd.memset(g_lhs, 0.0)
            nc.gpsimd.memset(g_rhs, 0.0)
            warm_ps = ppool0.tile([128, SC], F32, tag="ps0")
            for i in range(N_WARM):
                nc.tensor.matmul(
                    warm_ps, lhsT=g_lhs, rhs=g_rhs,
                    start=(i == 0), stop=(i == N_WARM - 1),
                )

            # Startup loads: ALL on the sync HWDGE ring, in the order the
            # PE consumes them (w0aa q0k01 w0ba b0 q0k23 w0ab w0bb q1 |
            # w1 b1 | w2 b2). One queue streaming large DMAs sustains
            # ~341-365 GB/s, while splitting the same bytes across parallel
            # rings drains slower aggregate — and FIFO order on one ring
            # guarantees chunk-0 data is never slowed by the later,
            # slack-rich w1/w2 transfers. With this order, pass A(m0-3)
            # needs only 0.75 MiB (~11.3 us) and every later pass's data
            # arrives before the PE reaches it.
            w0_sb = [[None, None], [None, None]]  # [khalf][mhalf]
            for kh in range(2):
                for mh in range(2):
                    w0_sb[kh][mh] = wpool.tile(
                        [128, 2, DH // 2], BF16, tag=f"w0_{kh}_{mh}",
                        name=f"w0_{kh}_{mh}",
                    )
            b0_sb = bpool.tile([128, M0], F32, tag="b0")
            b1_sb = bpool.tile([128, M0], F32, tag="b1")

            def w0_slice(k, m):
                t = w0_sb[k // 2][m // 4]
                return t[:, k % 2, (m % 4) * 128:(m % 4 + 1) * 128]

            def load_q(c):
                t = qpool.tile([128, K0, SC], BF16, tag="q", name=f"q{c}")
                nc.sync.dma_start(out=t, in_=qd[c])
                return t

            q0_sb = qpool.tile([128, K0, SC], BF16, tag="q", name="q0")
            nc.sync.dma_start(out=w0_sb[0][0], in_=w0q[0])
            nc.sync.dma_start(
                out=q0_sb[:, 0:K0 // 2, :], in_=qd[0][:, 0:(K0 // 2) * SC]
            )
            nc.sync.dma_start(out=w0_sb[1][0], in_=w0q[1])
            nc.sync.dma_start(out=b0_sb, in_=b0[:, :])
            nc.sync.dma_start(
                out=q0_sb[:, K0 // 2:K0, :], in_=qd[0][:, (K0 // 2) * SC:K0 * SC]
            )
            nc.sync.dma_start(out=w0_sb[0][1], in_=w0q[2])
            nc.sync.dma_start(out=w0_sb[1][1], in_=w0q[3])
            q1_sb = load_q(1)

            w1_sb = wpool.tile([128, K1, DH], BF16, tag="w1")
            nc.sync.dma_start(out=w1_sb, in_=w1d[:, :])
            nc.sync.dma_start(out=b1_sb, in_=b1[:, :])

            def w1_slice(k, m):
                return w1_sb[:, k, m * 128:(m + 1) * 128]

            w2_sb = wpool.tile([128, K1, DOUT], BF16, tag="w2")
            nc.sync.dma_start(out=w2_sb, in_=w2d[:, :])
            b2_sb = bpool.tile([128, DOUT], F32, tag="b2")
            b2_ap = b2[:]
            b2_bcast = bass.AP(
                tensor=b2_ap.tensor,
                offset=b2_ap.offset,
                ap=[[0, 128]] + [list(d) for d in b2_ap.ap],
            )
            nc.sync.dma_start(out=b2_sb, in_=b2_bcast)

            def layer0(c, q_sb):
                h0_sb = []
                for m in range(M0):
                    ps = ppool0.tile([128, SC], F32, tag="ps0", name=f"ps0_{c}_{m}")
                    for k in range(K0):
                        nc.tensor.matmul(
                            ps,
                            lhsT=w0_slice(k, m),
                            rhs=q_sb[:, k, :],
                            start=(k == 0),
                            stop=(k == K0 - 1),
                        )
                    h = apool.tile([128, SC], BF16, tag=f"h0_{m}", name=f"h0_{c}_{m}")
                    nc.scalar.activation(h, ps, Relu, bias=b0_sb[:, m:m + 1])
                    h0_sb.append(h)
                return h0_sb

            def layer0_c0(q_sb):
                # First chunk, k-split: each m-group's accumulation is
                # emitted as two k-pair passes with the PSUM group left
                # open in between, interleaved over 4 m-groups (= the 4
                # L0 banks). Pass A (k=0,1) needs only w0a+q0 (1.5 MiB,
                # ready ~12.3 us); w0b lands (~14 us) while pass A runs.
                # Full-width N=512 matmuls throughout (narrow warmups or
                # strips leave the clock governor at the mid p-state).
                h0_sb = [None] * M0
                for half in range(2):
                    ms = list(range(half * 4, half * 4 + 4))
                    pss = {}
                    for phase in range(2):
                        for m in ms:
                            if phase == 0:
                                pss[m] = ppool0.tile(
                                    [128, SC], F32, tag="ps0", name=f"ps0_0_{m}"
                                )
                            ps = pss[m]
                            for k in (phase * 2, phase * 2 + 1):
                                nc.tensor.matmul(
                                    ps,
                                    lhsT=w0_slice(k, m),
                                    rhs=q_sb[:, k, :],
                                    start=(k == 0),
                                    stop=(k == K0 - 1),
                                    skip_group_check=True,
                                )
                            if phase == 1:
                                h = apool.tile(
                                    [128, SC], BF16, tag=f"h0_{m}",
                                    name=f"h0_0_{m}",
                                )
                                nc.scalar.activation(
                                    h, ps, Relu, bias=b0_sb[:, m:m + 1]
                                )
                                h0_sb[m] = h
                return h0_sb

            def layer1(c, h0_sb):
                h1_sb = []
                for m in range(M0):
                    ps = ppool1.tile([128, SC], F32, tag="ps1", name=f"ps1_{c}_{m}")
                    for k in range(K1):
                        nc.tensor.matmul(
                            ps,
                            lhsT=w1_slice(k, m),
                            rhs=h0_sb[k],
                            start=(k == 0),
                            stop=(k == K1 - 1),
                        )
                    h = apool.tile([128, SC], BF16, tag=f"h1_{m}", name=f"h1_{c}_{m}")
                    nc.scalar.activation(h, ps, Relu, bias=b1_sb[:, m:m + 1])
                    h1_sb.append(h)
                return h1_sb

            def layer2(c, h1_sb):
                s0 = c * SC
                last = c == NCH - 1
                for mt in range(MT):
                    ps = ppool2.tile([128, DOUT], F32, tag="ps2", name=f"ps2_{c}_{mt}")
                    for k in range(K1):
                        nc.tensor.matmul(
                            ps,
                            lhsT=h1_sb[k][:, mt * 128:(mt + 1) * 128],
                            rhs=w2_sb[:, k, :],
                            start=(k == 0),
                            stop=(k == K1 - 1),
                        )
                    ot = opool.tile([128, DOUT], BF16, tag="ot", name=f"ot_{c}_{mt}")
                    r0 = s0 + mt * 128
                    if last and mt == MT - 1:
                        # Tail trim: quarter the strictly-serial
                        # PSUM->add->DMA chain after the very last matmul,
                        # alternating the two HWDGE rings so issue overlaps
                        # (halves were tried: 240.9 us min vs 240.3 — the
                        # longer first add outweighs the parallel issues).
                        Q = DOUT // 4
                        for i in range(4):
                            sl = slice(i * Q, (i + 1) * Q)
                            nc.vector.tensor_add(ot[:, sl], ps[:, sl], b2_sb[:, sl])
                            eng = nc.scalar if i % 2 == 0 else nc.sync
                            eng.dma_start(out=out[r0:r0 + 128, sl], in_=ot[:, sl])
                    else:
                        nc.vector.tensor_add(ot, ps, b2_sb)
                        eng = nc.scalar if mt % 2 == 0 else nc.sync
                        eng.dma_start(out=out[r0:r0 + 128, :], in_=ot)

            # Software pipeline: emit L0 of chunk c+1 ahead of L1/L2 of
            # chunk c, so the matmul stream never depends on a DMA issued
            # less than a full chunk earlier.
            h0_cur = layer0_c0(q0_sb)
            for c in range(NCH):
                h0_next = None
                if c + 1 < NCH:
                    h0_next = layer0(c + 1, q1_sb if c == 0 else load_q(c + 1))
                layer2(c, layer1(c, h0_cur))
                h0_cur = h0_next
    nc.finalize()
    return nc


_NC = None


def _get_nc():
    global _NC
    if _NC is None:
        _NC = build_nc()
    return _NC


def make_in_maps(inputs):
    bf16 = ml_dtypes.bfloat16
    q, W0, b0, W1, b1, W2, b2 = (
        inputs["query"], inputs["W0"], inputs["b0"], inputs["W1"],
        inputs["b1"], inputs["W2"], inputs["b2"],
    )
    in_maps = []
    for b in range(B):
        # qd[c, p, k*SC + s] = q[b, c*SC+s, k*128+p]
        qT = np.asarray(q[b]).T.astype(bf16)            # [DIN, S]
        qd = (
            qT.reshape(K0, 128, NCH, SC)
            .transpose(2, 1, 0, 3)
            .reshape(NCH, 128, K0 * SC)
        )

        # wXd[p, k*out + j] = WX[b].T[(k*128+p), j]
        w0t = np.asarray(W0[b]).T.astype(bf16)          # [DIN, DH]
        # w0q[mh*2+kh, p, kk*512 + col] = w0t[(kh*2+kk)*128 + p, mh*512 + col]
        w0q = (
            w0t.reshape(2, 2, 128, 2, DH // 2)          # [kh, kk, p, mh, col]
            .transpose(3, 0, 2, 1, 4)                   # [mh, kh, p, kk, col]
            .reshape(4, 128, DH)
        )
        w1t = np.asarray(W1[b]).T.astype(bf16)          # [DH, DH]
        w1d = w1t.reshape(K1, 128, DH).transpose(1, 0, 2).reshape(128, K1 * DH)
        w2t = np.asarray(W2[b]).T.astype(bf16)          # [DH, DOUT]
        w2d = w2t.reshape(K1, 128, DOUT).transpose(1, 0, 2).reshape(128, K1 * DOUT)
        in_maps.append({
            "qd": np.ascontiguousarray(qd),
            "w0q": np.ascontiguousarray(w0q),
            "w1d": np.ascontiguousarray(w1d),
            "w2d": np.ascontiguousarray(w2d),
            "b0": np.ascontiguousarray(
                np.asarray(b0[b], dtype=np.float32).reshape(DH // 128, 128).T
            ),
            "b1": np.ascontiguousarray(
                np.asarray(b1[b], dtype=np.float32).reshape(DH // 128, 128).T
            ),
            "b2": np.asarray(b2[b], dtype=np.float32),
        })
    return in_maps


def run(inputs, trace=False):
    nc = _get_nc()
    in_maps = make_in_maps(inputs)
    res = run_bass_kernel_spmd(nc, in_maps, core_ids=list(range(B)), trace=trace)
    out = np.stack(
        [np.asarray(r["out"]).astype(np.float32) for r in res.results]
    )
    return out, res


def kernel(**inputs) -> np.ndarray:
    out, _ = run(inputs, trace=False)
    return out
